# revision 9
# baseline (speedup 1.0000x reference)
"""3-layer GAT (graph attention network) on 8 Trainium2 NeuronCores.

Strategy: node-sharded graph parallelism.
- Nodes padded 10000 -> 10240, 1280 per core; edges partitioned by dst range.
- Per layer: each core computes table rows [h | es | ed] for its nodes with a
  PE matmul against W_ext = [W | W@Bsrc | W@Bdst] (bf16), AllGathers the full
  table, then processes its edges: dma_gather of h[src] rows, attention
  softmax without segment-max (exponents are bounded, softmax is shift
  invariant), and scatter-by-matmul: sel[e,dst] one-hot matrices contract
  128-edge blocks into per-dst-tile PSUM accumulators for both the
  numerator (sum alpha*h) and denominator (sum exp e).
"""

import numpy as np
import ml_dtypes

import concourse.bass as bass
import concourse.bacc as bacc
import concourse.mybir as mybir
import concourse.tile as tile
from concourse.library_config import mlp
from concourse.masks import make_identity
from concourse.bass_utils import run_bass_kernel_spmd
from concourse._compat import cdiv

F32 = mybir.dt.float32
DT = mybir.dt.bfloat16
NPDT = ml_dtypes.bfloat16

N, E, D = 10000, 160000, 512
H, C = 4, 128
HF, CF = 2, 512
NEG = 0.2
EPS = 1e-16

NCORES = 8
NPAD = 10240
NPC = NPAD // NCORES       # 1280 nodes per core
NTL = NPC // 128           # 10 local dst tiles per core
NTG = NPAD // 128          # 80 global node tiles
CB = 6                     # gather-chunk size in 128-edge blocks (768 idxs)
WT12 = 640                 # bf16 table row width, layers 1-2 (512+4+4 -> pad)
WT3 = 1152                 # layer 3 (1024+2+2 -> pad)

_cache = {}


def _block_diag(a):
    """[Hh, Cc] attention vector -> [Hh*Cc, Hh] block-diagonal embed."""
    Hh, Cc = a.shape
    B = np.zeros((Hh * Cc, Hh), np.float32)
    for h in range(Hh):
        B[h * Cc:(h + 1) * Cc, h] = a[h]
    return B


def _prep_host(graph, edge_index, W1, as1, ad1, b1, W2, as2, ad2, b2,
               W3, as3, ad3, b3):
    src = np.asarray(edge_index[0], np.int64)
    dst = np.asarray(edge_index[1], np.int64)

    dstt = dst // 128
    cnt = np.bincount(dstt, minlength=NTG)
    nb = int(np.ceil(cnt.max() / 128))
    NB = max(18, int(cdiv(nb, CB)) * CB)          # blocks per dst tile
    NCHUNK = NB // CB
    order = np.argsort(dstt, kind="stable")
    off = np.concatenate([[0], np.cumsum(cnt)])

    idx_slots = np.zeros((NTG, NB * 128), np.int16)
    dstl_slots = np.full((NTG, NB * 128), 255.0, np.float32)
    for gt in range(NTG):
        e = order[off[gt]:off[gt + 1]]
        k = len(e)
        idx_slots[gt, :k] = src[e].astype(np.int16)
        dstl_slots[gt, :k] = (dst[e] - gt * 128).astype(np.float32)

    # dma_gather wrapped index layout: within each 768-idx chunk,
    # unwrapped[j] = wrapped[j % 16, j // 16]; replicated to 128 partitions.
    w = idx_slots.reshape(NTG, NCHUNK, 48, 16).transpose(0, 1, 3, 2)  # [.,.,16,48]
    w = np.tile(w, (1, 1, 8, 1))                                      # [.,.,128,48]
    # dstl column layout: [p, b] = slot b*128+p
    dcol = dstl_slots.reshape(NTG, NB, 128).transpose(0, 2, 1)        # [NTG,128,NB]

    xpad = np.zeros((NPAD, D), np.float32)
    xpad[:N] = np.asarray(graph, np.float32)

    def wext(W, a_s, a_d, wt):
        cols = np.concatenate(
            [W, W @ _block_diag(a_s), W @ _block_diag(a_d)], axis=1)
        out = np.zeros((D, wt), np.float32)
        out[:, :cols.shape[1]] = cols
        return out.astype(NPDT)

    we1 = wext(np.asarray(W1, np.float32), np.asarray(as1), np.asarray(ad1), WT12)
    we2 = wext(np.asarray(W2, np.float32), np.asarray(as2), np.asarray(ad2), WT12)
    we3 = wext(np.asarray(W3, np.float32), np.asarray(as3), np.asarray(ad3), WT3)

    iotaF6 = np.tile(np.arange(128, dtype=np.float32)[None, :], (128, 6)).astype(NPDT)
    iotaP = np.arange(128, dtype=np.float32)[:, None].astype(NPDT)
    ones_row = np.ones((1, 128), np.float32)

    in_maps = []
    for c in range(NCORES):
        tl = slice(c * NTL, (c + 1) * NTL)
        idx_c = w[tl].transpose(2, 0, 1, 3).reshape(128, NTL * NCHUNK * 48)
        dstl_c = dcol[tl].transpose(1, 0, 2).reshape(128, NTL * NB).astype(NPDT)
        dstlb_c = np.tile(dstl_slots[tl].reshape(1, NTL * NB * 128),
                          (128, 1)).astype(NPDT)
        xgT_c = np.ascontiguousarray(
            xpad[c * NPC:(c + 1) * NPC].T).astype(NPDT)   # [512, 1280]
        xl_c = xpad[c * NPC:(c + 1) * NPC]                 # [1280, 512] f32
        in_maps.append({
            "idx": np.ascontiguousarray(idx_c),
            "dstl": np.ascontiguousarray(dstl_c),
            "dstlb": np.ascontiguousarray(dstlb_c),
            "xgT": xgT_c,
            "xl": np.ascontiguousarray(xl_c),
            "we1": we1, "we2": we2, "we3": we3,
            "b1": np.asarray(b1, np.float32)[None, :],
            "b2": np.asarray(b2, np.float32)[None, :],
            "b3": np.asarray(b3, np.float32)[None, :],
            "iotaF6": iotaF6,
            "iotaP": iotaP,
            "ones": ones_row,
        })
    return in_maps, NB


def _build_nc(NB):
    NCHUNK = NB // CB
    nc = bacc.Bacc("TRN2", target_bir_lowering=False, debug=False,
                   num_devices=NCORES, num_swdge_queues=4)

    t_idx = nc.dram_tensor("idx", [128, NTL * NCHUNK * 48], mybir.dt.int16,
                           kind="ExternalInput")
    t_dstl = nc.dram_tensor("dstl", [128, NTL * NB], DT, kind="ExternalInput")
    t_dstlb = nc.dram_tensor("dstlb", [128, NTL * NB * 128], DT,
                             kind="ExternalInput")
    t_xgT = nc.dram_tensor("xgT", [D, NPC], DT, kind="ExternalInput")
    t_xl = nc.dram_tensor("xl", [NPC, D], F32, kind="ExternalInput")
    t_we = {1: nc.dram_tensor("we1", [D, WT12], DT, kind="ExternalInput"),
            2: nc.dram_tensor("we2", [D, WT12], DT, kind="ExternalInput"),
            3: nc.dram_tensor("we3", [D, WT3], DT, kind="ExternalInput")}
    t_b = {1: nc.dram_tensor("b1", [1, D], F32, kind="ExternalInput"),
           2: nc.dram_tensor("b2", [1, D], F32, kind="ExternalInput"),
           3: nc.dram_tensor("b3", [1, D], F32, kind="ExternalInput")}
    t_iotaF6 = nc.dram_tensor("iotaF6", [128, 768], DT, kind="ExternalInput")
    t_iotaP = nc.dram_tensor("iotaP", [128, 1], DT, kind="ExternalInput")
    t_ones = nc.dram_tensor("ones", [1, 128], F32, kind="ExternalInput")
    t_out = nc.dram_tensor("out", [NPC, D], F32, kind="ExternalOutput")

    rg = [list(range(NCORES))]
    qn = [0]

    with tile.TileContext(nc) as tc:
        with tc.tile_pool(name="cst", bufs=1) as cst, \
             tc.tile_pool(name="per", bufs=1) as per, \
             tc.tile_pool(name="wk", bufs=2) as wk, \
             tc.tile_pool(name="ed", bufs=2) as edp, \
             tc.tile_pool(name="gath", bufs=3) as gp, \
             tc.tile_pool(name="pnum", bufs=2, space="PSUM") as pnum, \
             tc.tile_pool(name="pden", bufs=2, space="PSUM") as pden, \
             tc.tile_pool(name="psmall", bufs=2, space="PSUM") as psml, \
             tc.tile_pool(name="dram", bufs=1, space="DRAM") as dram:

            nc.gpsimd.load_library(mlp)

            # ---- constants -------------------------------------------------
            iotaF6 = cst.tile([128, 768], DT)
            nc.sync.dma_start(iotaF6[:], t_iotaF6[:])
            iotaP = cst.tile([128, 1], DT)
            nc.sync.dma_start(iotaP[:], t_iotaP[:])
            id_f32 = cst.tile([128, 128], F32)
            make_identity(nc, id_f32[:])
            ones_t = cst.tile([1, 128], F32)
            nc.sync.dma_start(ones_t[:], t_ones[:])

            idx_t = per.tile([128, NTL * NCHUNK * 48], mybir.dt.int16)
            nc.sync.dma_start(idx_t[:], t_idx[:])
            dstl = per.tile([128, NTL * NB], DT)
            nc.sync.dma_start(dstl[:], t_dstl[:])

            b_bc = {}
            for l in (1, 2, 3):
                br = wk.tile([1, D], F32, tag="brow", bufs=1)
                nc.sync.dma_start(br[:], t_b[l][:])
                pb = pnum.tile([128, D], F32, space="PSUM", tag="numA")
                nc.tensor.matmul(pb[:], lhsT=ones_t[:], rhs=br[:],
                                 start=True, stop=True)
                b_bc[l] = cst.tile([128, D], F32, tag=f"bbc{l}", name=f"bbc{l}")
                nc.vector.tensor_copy(b_bc[l][:], pb[:])

            we = {}
            for l in (1, 2, 3):
                wt = WT3 if l == 3 else WT12
                we[l] = per.tile([128, 4, wt], DT, tag=f"we{l}", name=f"we{l}")
                nc.sync.dma_start(
                    we[l][:],
                    t_we[l][:].rearrange("(kb p) w -> p kb w", p=128))

            # persistent x buffers (f32) for layer 2/3 inputs
            xA = per.tile([128, NTL, D], F32, tag="xA")
            xB = per.tile([128, NTL, D], F32, tag="xB")

            # ---- per-layer helpers ----------------------------------------
            def build_table(l, tbl_in, get_lhsT):
                """Local table rows: [h | es | ed] for this core's nodes."""
                wt = WT3 if l == 3 else WT12
                hw = HF * CF if l == 3 else H * C
                nh = HF if l == 3 else H
                segs = [(0, 512), (512, 1024), (1024, 1024 + 2 * nh)] if l == 3 \
                    else [(0, 512), (512, 512 + 2 * nh)]
                for nt in range(NTL):
                    lhsT = get_lhsT(nt)  # [128, 4, 128] DT tile
                    psums = []
                    for si, (c0, c1) in enumerate(segs):
                        if c1 - c0 > 64:
                            pool_, tag = pnum, ("numA" if si == 0 else "numB")
                        else:
                            pool_, tag = psml, "small"
                        p = pool_.tile([128, c1 - c0], F32, space="PSUM",
                                       tag=tag, name=f"p_tbl{si}")
                        for kb in range(4):
                            nc.tensor.matmul(p[:], lhsT=lhsT[:, kb, :],
                                             rhs=we[l][:, kb, c0:c1],
                                             start=(kb == 0), stop=(kb == 3))
                        psums.append((c0, c1, p))
                    row = wk.tile([128, wt], DT, tag="tblrow")
                    for c0, c1, p in psums:
                        if c1 - c0 > 64:
                            nc.scalar.activation(
                                row[:, c0:c1], p[:],
                                mybir.ActivationFunctionType.Copy)
                        else:
                            nc.vector.tensor_copy(row[:, c0:c1], p[:])
                    nc.sync.dma_start(tbl_in[nt * 128:(nt + 1) * 128, :], row[:])

            def edge_phase(l, tbl, tbl_in, x_prev, x_next):
                wt = WT3 if l == 3 else WT12
                nh = HF if l == 3 else H
                ch = CF if l == 3 else C
                hw = nh * ch
                es_off, ed_off = hw, hw + nh
                for t in range(NTL):
                    dstlb_t = wk.tile([128, NB * 128], DT, tag="dstlb")
                    nc.sync.dma_start(
                        dstlb_t[:],
                        t_dstlb[:, t * NB * 128:(t + 1) * NB * 128])
                    ed_t = edp.tile([128, nh], DT, tag="edt")
                    nc.sync.dma_start(
                        ed_t[:],
                        tbl_in[t * 128:(t + 1) * 128, ed_off:ed_off + nh])
                    if l == 3:
                        p_num0 = pnum.tile([128, 512], F32, space="PSUM", tag="numA")
                        p_num1 = pnum.tile([128, 512], F32, space="PSUM", tag="numB")
                    else:
                        p_num = pnum.tile([128, 512], F32, space="PSUM", tag="numA")
                    p_den = pden.tile([128, nh], F32, space="PSUM", tag="den")
                    for c in range(NB // CB):
                        gt = gp.tile([128, CB, wt], DT, tag="gt")
                        icol = (t * (NB // CB) + c) * 48
                        nc.gpsimd.dma_gather(
                            gt[:], tbl[:], idx_t[:, icol:icol + 48],
                            CB * 128, CB * 128, wt, queue_num=qn[0] % 4)
                        qn[0] += 1
                        scol = c * CB * 128
                        selT = wk.tile([128, CB * 128], DT, tag="selT")
                        nc.vector.tensor_tensor(
                            out=selT[:], in0=iotaP[:].to_broadcast([128, CB * 128]),
                            in1=dstlb_t[:, scol:scol + CB * 128],
                            op=mybir.AluOpType.is_equal)
                        sel = wk.tile([128, CB, 128], DT, tag="sel")
                        dc0 = t * NB + c * CB
                        nc.vector.tensor_tensor(
                            out=sel[:],
                            in0=dstl[:, dc0:dc0 + CB, None].to_broadcast(
                                [128, CB, 128]),
                            in1=iotaF6[:].rearrange("p (b f) -> p b f", b=CB),
                            op=mybir.AluOpType.is_equal)
                        p_ede = psml.tile([128, CB * nh], F32, space="PSUM",
                                          tag="small")
                        for j in range(CB):
                            nc.tensor.matmul(
                                p_ede[:, j * nh:(j + 1) * nh],
                                lhsT=selT[:, j * 128:(j + 1) * 128],
                                rhs=ed_t[:], start=True, stop=True)
                        e0 = wk.tile([128, CB * nh], F32, tag="e0")
                        nc.vector.tensor_tensor(
                            out=e0[:].rearrange("p (b h) -> p b h", b=CB),
                            in0=gt[:, :, es_off:es_off + nh],
                            in1=p_ede[:].rearrange("p (b h) -> p b h", b=CB),
                            op=mybir.AluOpType.add)
                        e1 = wk.tile([128, CB * nh], F32, tag="e1")
                        nc.vector.tensor_scalar_mul(e1[:], e0[:], NEG)
                        e2 = wk.tile([128, CB * nh], F32, tag="e2")
                        nc.vector.tensor_tensor(out=e2[:], in0=e0[:], in1=e1[:],
                                                op=mybir.AluOpType.max)
                        exf = wk.tile([128, CB * nh], F32, tag="exf")
                        nc.scalar.activation(exf[:], e2[:],
                                             mybir.ActivationFunctionType.Exp)
                        ex = wk.tile([128, CB * nh], DT, tag="ex")
                        nc.vector.tensor_copy(ex[:], exf[:])
                        for j in range(CB):
                            b = c * CB + j
                            first, last = (b == 0), (b == NB - 1)
                            msg = wk.tile([128, hw], DT, tag="msg")
                            if l == 3:
                                nc.scalar.activation(
                                    msg[:, 0:ch], gt[:, j, 0:ch],
                                    mybir.ActivationFunctionType.Copy,
                                    scale=exf[:, j * nh:j * nh + 1])
                                nc.vector.tensor_tensor(
                                    out=msg[:, ch:2 * ch],
                                    in0=gt[:, j, ch:2 * ch],
                                    in1=ex[:, j * nh + 1:j * nh + 2]
                                        .to_broadcast([128, ch]),
                                    op=mybir.AluOpType.mult)
                            else:
                                nc.vector.tensor_tensor(
                                    out=msg[:].rearrange("p (h c) -> p h c", h=nh),
                                    in0=gt[:, j, 0:hw].rearrange(
                                        "p (h c) -> p h c", h=nh),
                                    in1=ex[:, j * nh:(j + 1) * nh]
                                        .to_broadcast([128, nh, ch]),
                                    op=mybir.AluOpType.mult)
                            nc.tensor.matmul(p_den[:],
                                             lhsT=sel[:, j, :],
                                             rhs=ex[:, j * nh:(j + 1) * nh],
                                             start=first, stop=last)
                            if l == 3:
                                nc.tensor.matmul(p_num0[:], lhsT=sel[:, j, :],
                                                 rhs=msg[:, 0:512],
                                                 start=first, stop=last)
                                nc.tensor.matmul(p_num1[:], lhsT=sel[:, j, :],
                                                 rhs=msg[:, 512:1024],
                                                 start=first, stop=last)
                            else:
                                nc.tensor.matmul(p_num[:], lhsT=sel[:, j, :],
                                                 rhs=msg[:],
                                                 start=first, stop=last)
                    # tile epilogue
                    dn = wk.tile([128, nh], F32, tag="dn", bufs=1)
                    nc.vector.tensor_scalar_add(dn[:], p_den[:], EPS)
                    rc = wk.tile([128, nh], F32, tag="rc", bufs=1)
                    nc.vector.reciprocal(rc[:], dn[:])
                    if l == 3:
                        a0 = wk.tile([128, 512], F32, tag="a0", bufs=1)
                        nc.vector.tensor_tensor(
                            out=a0[:], in0=p_num0[:],
                            in1=rc[:, 0:1].to_broadcast([128, 512]),
                            op=mybir.AluOpType.mult)
                        a1 = wk.tile([128, 512], F32, tag="a1", bufs=1)
                        nc.vector.tensor_tensor(
                            out=a1[:], in0=p_num1[:],
                            in1=rc[:, 1:2].to_broadcast([128, 512]),
                            op=mybir.AluOpType.mult)
                        s0 = wk.tile([128, 512], F32, tag="s0", bufs=1)
                        nc.vector.tensor_tensor(out=s0[:], in0=a0[:], in1=a1[:],
                                                op=mybir.AluOpType.add)
                        s1 = wk.tile([128, 512], F32, tag="s1", bufs=1)
                        nc.vector.tensor_scalar(
                            out=s1[:], in0=s0[:], scalar1=0.5, scalar2=None,
                            op0=mybir.AluOpType.mult)
                        s2 = wk.tile([128, 512], F32, tag="s2", bufs=1)
                        nc.vector.tensor_tensor(out=s2[:], in0=s1[:],
                                                in1=x_prev(t),
                                                op=mybir.AluOpType.add)
                        s3 = wk.tile([128, 512], F32, tag="s3", bufs=1)
                        nc.vector.tensor_tensor(out=s3[:], in0=s2[:],
                                                in1=b_bc[3][:],
                                                op=mybir.AluOpType.add)
                        nc.sync.dma_start(t_out[t * 128:(t + 1) * 128, :], s3[:])
                    else:
                        agg = wk.tile([128, 512], F32, tag="agg", bufs=1)
                        nc.vector.tensor_tensor(
                            out=agg[:].rearrange("p (h c) -> p h c", h=nh),
                            in0=p_num[:].rearrange("p (h c) -> p h c", h=nh),
                            in1=rc[:].to_broadcast([128, nh, ch]),
                            op=mybir.AluOpType.mult)
                        s0 = wk.tile([128, 512], F32, tag="s0", bufs=1)
                        nc.vector.tensor_tensor(out=s0[:], in0=agg[:],
                                                in1=x_prev(t),
                                                op=mybir.AluOpType.add)
                        s1 = wk.tile([128, 512], F32, tag="s1", bufs=1)
                        nc.vector.tensor_tensor(out=s1[:], in0=s0[:],
                                                in1=b_bc[l][:],
                                                op=mybir.AluOpType.add)
                        # elu(x) = max(x,0) + exp(min(x,0)) - 1
                        mn = wk.tile([128, 512], F32, tag="mn", bufs=1)
                        nc.vector.tensor_scalar_min(mn[:], s1[:], 0.0)
                        ep = wk.tile([128, 512], F32, tag="ep", bufs=1)
                        nc.scalar.activation(ep[:], mn[:],
                                             mybir.ActivationFunctionType.Exp)
                        mx = wk.tile([128, 512], F32, tag="mx", bufs=1)
                        nc.vector.tensor_scalar(out=mx[:], in0=s1[:], scalar1=0.0,
                                                scalar2=-1.0,
                                                op0=mybir.AluOpType.max,
                                                op1=mybir.AluOpType.add)
                        nc.vector.tensor_tensor(out=x_next[:, t, :], in0=mx[:],
                                                in1=ep[:],
                                                op=mybir.AluOpType.add)

            # ---- layer 1 ---------------------------------------------------
            tbl1_in = dram.tile([NPC, WT12], DT, tag="t1in")
            tbl1 = dram.tile([NPAD, WT12], DT, tag="t1", addr_space="Shared")

            def lhsT_l1(nt):
                x_nt = wk.tile([128, 4, 128], DT, tag="xnt")
                nc.sync.dma_start(
                    x_nt[:],
                    t_xgT[:].rearrange("(kb p) n -> p kb n", p=128)
                        [:, :, nt * 128:(nt + 1) * 128])
                return x_nt

            build_table(1, tbl1_in, lhsT_l1)
            nc.gpsimd.collective_compute(
                "AllGather", mybir.AluOpType.bypass, replica_groups=rg,
                ins=[tbl1_in[:]], outs=[tbl1[:]])

            def xprev1(t):
                xp = wk.tile([128, 512], F32, tag="xp1", bufs=1)
                nc.sync.dma_start(xp[:], t_xl[t * 128:(t + 1) * 128, :])
                return xp[:]

            edge_phase(1, tbl1, tbl1_in, xprev1, xA)

            # ---- layers 2, 3 ----------------------------------------------
            def transpose_lhsT(x_buf):
                def get(nt):
                    xt = wk.tile([128, 4, 128], DT, tag="xnt")
                    for kb in range(4):
                        p_t = psml.tile([128, 128], F32, space="PSUM", tag="small")
                        nc.tensor.transpose(
                            out=p_t[:],
                            in_=x_buf[:, nt, kb * 128:(kb + 1) * 128],
                            identity=id_f32[:])
                        nc.vector.tensor_copy(xt[:, kb, :], p_t[:])
                    return xt
                return get

            tbl2_in = dram.tile([NPC, WT12], DT, tag="t2in")
            tbl2 = dram.tile([NPAD, WT12], DT, tag="t2", addr_space="Shared")
            build_table(2, tbl2_in, transpose_lhsT(xA))
            nc.gpsimd.collective_compute(
                "AllGather", mybir.AluOpType.bypass, replica_groups=rg,
                ins=[tbl2_in[:]], outs=[tbl2[:]])
            edge_phase(2, tbl2, tbl2_in, lambda t: xA[:, t, :], xB)

            tbl3_in = dram.tile([NPC, WT3], DT, tag="t3in")
            tbl3 = dram.tile([NPAD, WT3], DT, tag="t3", addr_space="Shared")
            build_table(3, tbl3_in, transpose_lhsT(xB))
            nc.gpsimd.collective_compute(
                "AllGather", mybir.AluOpType.bypass, replica_groups=rg,
                ins=[tbl3_in[:]], outs=[tbl3[:]])
            edge_phase(3, tbl3, tbl3_in, lambda t: xB[:, t, :], None)

    nc.compile()
    return nc


def _run(inputs, trace=False):
    in_maps, NB = _prep_host(**inputs)
    key = NB
    if key not in _cache:
        _cache[key] = _build_nc(NB)
    nc = _cache[key]
    res = run_bass_kernel_spmd(nc, in_maps, core_ids=list(range(NCORES)),
                               trace=trace)
    out = np.concatenate([r["out"] for r in res.results], axis=0)[:N]
    return out, res


def kernel(**inputs):
    out, _ = _run(inputs, trace=False)
    return out


# revision 10
# speedup vs baseline: 1.1585x; 1.1585x over previous
"""3-layer GAT (graph attention network) on 8 Trainium2 NeuronCores.

Strategy: node-sharded graph parallelism.
- Nodes padded 10000 -> 10240, 1280 per core; edges partitioned by dst range.
- Per layer: each core computes table rows [h | es | ed] for its nodes with a
  PE matmul against W_ext = [W | W@Bsrc | W@Bdst] (bf16), AllGathers the full
  table, then processes its edges: dma_gather of h[src] rows, attention
  softmax without segment-max (exponents are bounded, softmax is shift
  invariant), and scatter-by-matmul: sel[e,dst] one-hot matrices contract
  128-edge blocks into per-dst-tile PSUM accumulators for both the
  numerator (sum alpha*h) and denominator (sum exp e).
"""

import numpy as np
import ml_dtypes

import concourse.bass as bass
import concourse.bacc as bacc
import concourse.mybir as mybir
import concourse.tile as tile
from concourse.library_config import mlp
from concourse.masks import make_identity
from concourse.bass_utils import run_bass_kernel_spmd
from concourse._compat import cdiv

F32 = mybir.dt.float32
DT = mybir.dt.bfloat16
NPDT = ml_dtypes.bfloat16

N, E, D = 10000, 160000, 512
H, C = 4, 128
HF, CF = 2, 512
NEG = 0.2
EPS = 1e-16

NCORES = 8
NPAD = 10240
NPC = NPAD // NCORES       # 1280 nodes per core
NTL = NPC // 128           # 10 local dst tiles per core
NTG = NPAD // 128          # 80 global node tiles
CB = 6                     # gather-chunk size in 128-edge blocks (768 idxs)
WT12 = 640                 # bf16 table row width, layers 1-2 (512+4+4 -> pad)
WT3 = 1152                 # layer 3 (1024+2+2 -> pad)

_cache = {}


def _block_diag(a):
    """[Hh, Cc] attention vector -> [Hh*Cc, Hh] block-diagonal embed."""
    Hh, Cc = a.shape
    B = np.zeros((Hh * Cc, Hh), np.float32)
    for h in range(Hh):
        B[h * Cc:(h + 1) * Cc, h] = a[h]
    return B


def _prep_host(graph, edge_index, W1, as1, ad1, b1, W2, as2, ad2, b2,
               W3, as3, ad3, b3):
    src = np.asarray(edge_index[0], np.int64)
    dst = np.asarray(edge_index[1], np.int64)

    dstt = dst // 128
    cnt = np.bincount(dstt, minlength=NTG)
    nb = int(np.ceil(cnt.max() / 128))
    NB = max(18, int(cdiv(nb, CB)) * CB)          # blocks per dst tile
    NCHUNK = NB // CB
    order = np.argsort(dstt, kind="stable")
    off = np.concatenate([[0], np.cumsum(cnt)])

    idx_slots = np.zeros((NTG, NB * 128), np.int16)
    dstl_slots = np.full((NTG, NB * 128), 255.0, np.float32)
    for gt in range(NTG):
        e = order[off[gt]:off[gt + 1]]
        k = len(e)
        idx_slots[gt, :k] = src[e].astype(np.int16)
        dstl_slots[gt, :k] = (dst[e] - gt * 128).astype(np.float32)

    # dma_gather wrapped index layout: within each 768-idx chunk,
    # unwrapped[j] = wrapped[j % 16, j // 16]; replicated to 128 partitions.
    w = idx_slots.reshape(NTG, NCHUNK, 48, 16).transpose(0, 1, 3, 2)  # [.,.,16,48]
    w = np.tile(w, (1, 1, 8, 1))                                      # [.,.,128,48]
    # dstl column layout: [p, b] = slot b*128+p
    dcol = dstl_slots.reshape(NTG, NB, 128).transpose(0, 2, 1)        # [NTG,128,NB]

    xpad = np.zeros((NPAD, D), np.float32)
    xpad[:N] = np.asarray(graph, np.float32)

    def wext(W, a_s, a_d, wt):
        cols = np.concatenate(
            [W, W @ _block_diag(a_s), W @ _block_diag(a_d)], axis=1)
        out = np.zeros((D, wt), np.float32)
        out[:, :cols.shape[1]] = cols
        return out.astype(NPDT)

    we1 = wext(np.asarray(W1, np.float32), np.asarray(as1), np.asarray(ad1), WT12)
    we2 = wext(np.asarray(W2, np.float32), np.asarray(as2), np.asarray(ad2), WT12)
    we3 = wext(np.asarray(W3, np.float32), np.asarray(as3), np.asarray(ad3), WT3)

    iotaF6 = np.tile(np.arange(128, dtype=np.float32)[None, :], (128, 6)).astype(NPDT)
    iotaP = np.arange(128, dtype=np.float32)[:, None].astype(NPDT)
    ones_row = np.ones((1, 128), np.float32)

    in_maps = []
    for c in range(NCORES):
        tl = slice(c * NTL, (c + 1) * NTL)
        idx_c = w[tl].transpose(2, 0, 1, 3).reshape(128, NTL * NCHUNK * 48)
        dstl_c = dcol[tl].transpose(1, 0, 2).reshape(128, NTL * NB).astype(NPDT)
        dstlb_c = np.tile(dstl_slots[tl].reshape(1, NTL * NB * 128),
                          (128, 1)).astype(NPDT)
        xgT_c = np.ascontiguousarray(
            xpad[c * NPC:(c + 1) * NPC].T).astype(NPDT)   # [512, 1280]
        xl_c = xpad[c * NPC:(c + 1) * NPC]                 # [1280, 512] f32
        in_maps.append({
            "idx": np.ascontiguousarray(idx_c),
            "dstl": np.ascontiguousarray(dstl_c),
            "dstlb": np.ascontiguousarray(dstlb_c),
            "xgT": xgT_c,
            "xl": np.ascontiguousarray(xl_c),
            "we1": we1, "we2": we2, "we3": we3,
            "b1": np.asarray(b1, np.float32)[None, :],
            "b2": np.asarray(b2, np.float32)[None, :],
            "b3": np.asarray(b3, np.float32)[None, :],
            "iotaF6": iotaF6,
            "iotaP": iotaP,
            "ones": ones_row,
        })
    return in_maps, NB


def _build_nc(NB):
    NCHUNK = NB // CB
    nc = bacc.Bacc("TRN2", target_bir_lowering=False, debug=False,
                   num_devices=NCORES, num_swdge_queues=4)

    t_idx = nc.dram_tensor("idx", [128, NTL * NCHUNK * 48], mybir.dt.int16,
                           kind="ExternalInput")
    t_dstl = nc.dram_tensor("dstl", [128, NTL * NB], DT, kind="ExternalInput")
    t_dstlb = nc.dram_tensor("dstlb", [128, NTL * NB * 128], DT,
                             kind="ExternalInput")
    t_xgT = nc.dram_tensor("xgT", [D, NPC], DT, kind="ExternalInput")
    t_xl = nc.dram_tensor("xl", [NPC, D], F32, kind="ExternalInput")
    t_we = {1: nc.dram_tensor("we1", [D, WT12], DT, kind="ExternalInput"),
            2: nc.dram_tensor("we2", [D, WT12], DT, kind="ExternalInput"),
            3: nc.dram_tensor("we3", [D, WT3], DT, kind="ExternalInput")}
    t_b = {1: nc.dram_tensor("b1", [1, D], F32, kind="ExternalInput"),
           2: nc.dram_tensor("b2", [1, D], F32, kind="ExternalInput"),
           3: nc.dram_tensor("b3", [1, D], F32, kind="ExternalInput")}
    t_iotaF6 = nc.dram_tensor("iotaF6", [128, 768], DT, kind="ExternalInput")
    t_iotaP = nc.dram_tensor("iotaP", [128, 1], DT, kind="ExternalInput")
    t_ones = nc.dram_tensor("ones", [1, 128], F32, kind="ExternalInput")
    t_out = nc.dram_tensor("out", [NPC, D], F32, kind="ExternalOutput")

    rg = [list(range(NCORES))]
    qn = [0]

    with tile.TileContext(nc) as tc:
        with tc.tile_pool(name="cst", bufs=1) as cst, \
             tc.tile_pool(name="per", bufs=1) as per, \
             tc.tile_pool(name="wk", bufs=2) as wk, \
             tc.tile_pool(name="ed", bufs=2) as edp, \
             tc.tile_pool(name="gath", bufs=3) as gp, \
             tc.tile_pool(name="pnum", bufs=2, space="PSUM") as pnum, \
             tc.tile_pool(name="pden", bufs=2, space="PSUM") as pden, \
             tc.tile_pool(name="psmall", bufs=2, space="PSUM") as psml, \
             tc.tile_pool(name="dram", bufs=1, space="DRAM") as dram:

            nc.gpsimd.load_library(mlp)

            # ---- constants -------------------------------------------------
            iotaF6 = cst.tile([128, 768], DT)
            nc.sync.dma_start(iotaF6[:], t_iotaF6[:])
            iotaP = cst.tile([128, 1], DT)
            nc.sync.dma_start(iotaP[:], t_iotaP[:])
            id_f32 = cst.tile([128, 128], F32)
            make_identity(nc, id_f32[:])
            ones_t = cst.tile([1, 128], F32)
            nc.sync.dma_start(ones_t[:], t_ones[:])

            idx_t = per.tile([128, NTL * NCHUNK * 48], mybir.dt.int16)
            nc.sync.dma_start(idx_t[:], t_idx[:])
            dstl = per.tile([128, NTL * NB], DT)
            nc.sync.dma_start(dstl[:], t_dstl[:])

            b_bc = {}
            for l in (1, 2, 3):
                br = wk.tile([1, D], F32, tag="brow", bufs=1)
                nc.sync.dma_start(br[:], t_b[l][:])
                pb = pnum.tile([128, D], F32, space="PSUM", tag="numA")
                nc.tensor.matmul(pb[:], lhsT=ones_t[:], rhs=br[:],
                                 start=True, stop=True)
                b_bc[l] = cst.tile([128, D], F32, tag=f"bbc{l}", name=f"bbc{l}")
                nc.vector.tensor_copy(b_bc[l][:], pb[:])

            we = {}
            for l in (1, 2, 3):
                wt = WT3 if l == 3 else WT12
                we[l] = per.tile([128, 4, wt], DT, tag=f"we{l}", name=f"we{l}")
                nc.sync.dma_start(
                    we[l][:],
                    t_we[l][:].rearrange("(kb p) w -> p kb w", p=128))

            # persistent x buffers (f32) for layer 2/3 inputs
            xA = per.tile([128, NTL, D], F32, tag="xA")
            xB = per.tile([128, NTL, D], F32, tag="xB")

            # ---- per-layer helpers ----------------------------------------
            def build_table(l, tbl_in, get_lhsT):
                """Local table rows: [h | es | ed] for this core's nodes."""
                wt = WT3 if l == 3 else WT12
                hw = HF * CF if l == 3 else H * C
                nh = HF if l == 3 else H
                segs = [(0, 512), (512, 1024), (1024, 1024 + 2 * nh)] if l == 3 \
                    else [(0, 512), (512, 512 + 2 * nh)]
                for nt in range(NTL):
                    lhsT = get_lhsT(nt)  # [128, 4, 128] DT tile
                    psums = []
                    for si, (c0, c1) in enumerate(segs):
                        if c1 - c0 > 64:
                            pool_, tag = pnum, ("numA" if si == 0 else "numB")
                        else:
                            pool_, tag = psml, "small"
                        p = pool_.tile([128, c1 - c0], F32, space="PSUM",
                                       tag=tag, name=f"p_tbl{si}")
                        for kb in range(4):
                            nc.tensor.matmul(p[:], lhsT=lhsT[:, kb, :],
                                             rhs=we[l][:, kb, c0:c1],
                                             start=(kb == 0), stop=(kb == 3))
                        psums.append((c0, c1, p))
                    row = wk.tile([128, wt], DT, tag="tblrow")
                    for c0, c1, p in psums:
                        nc.vector.tensor_copy(row[:, c0:c1], p[:])
                    nc.sync.dma_start(tbl_in[nt * 128:(nt + 1) * 128, :], row[:])

            def edge_phase(l, tbl, tbl_in, x_prev, x_next):
                wt = WT3 if l == 3 else WT12
                nh = HF if l == 3 else H
                ch = CF if l == 3 else C
                hw = nh * ch
                es_off, ed_off = hw, hw + nh
                for t in range(NTL):
                    dstlb_t = wk.tile([128, NB * 128], DT, tag="dstlb")
                    nc.sync.dma_start(
                        dstlb_t[:],
                        t_dstlb[:, t * NB * 128:(t + 1) * NB * 128])
                    ed_t = edp.tile([128, nh], DT, tag="edt")
                    nc.sync.dma_start(
                        ed_t[:],
                        tbl_in[t * 128:(t + 1) * 128, ed_off:ed_off + nh])
                    if l == 3:
                        p_num0 = pnum.tile([128, 512], F32, space="PSUM", tag="numA")
                        p_num1 = pnum.tile([128, 512], F32, space="PSUM", tag="numB")
                    else:
                        p_num = pnum.tile([128, 512], F32, space="PSUM", tag="numA")
                    p_den = pden.tile([128, nh], F32, space="PSUM", tag="den")
                    for c in range(NB // CB):
                        gt = gp.tile([128, CB, wt], DT, tag="gt")
                        icol = (t * (NB // CB) + c) * 48
                        nc.gpsimd.dma_gather(
                            gt[:], tbl[:], idx_t[:, icol:icol + 48],
                            CB * 128, CB * 128, wt, queue_num=qn[0] % 4)
                        qn[0] += 1
                        scol = c * CB * 128
                        selT = wk.tile([128, CB * 128], DT, tag="selT")
                        nc.vector.tensor_tensor(
                            out=selT[:], in0=iotaP[:].to_broadcast([128, CB * 128]),
                            in1=dstlb_t[:, scol:scol + CB * 128],
                            op=mybir.AluOpType.is_equal)
                        sel = wk.tile([128, CB, 128], DT, tag="sel")
                        dc0 = t * NB + c * CB
                        nc.vector.tensor_tensor(
                            out=sel[:],
                            in0=dstl[:, dc0:dc0 + CB, None].to_broadcast(
                                [128, CB, 128]),
                            in1=iotaF6[:].rearrange("p (b f) -> p b f", b=CB),
                            op=mybir.AluOpType.is_equal)
                        p_ede = psml.tile([128, CB * nh], F32, space="PSUM",
                                          tag="small")
                        for j in range(CB):
                            nc.tensor.matmul(
                                p_ede[:, j * nh:(j + 1) * nh],
                                lhsT=selT[:, j * 128:(j + 1) * 128],
                                rhs=ed_t[:], start=True, stop=True)
                        e0 = wk.tile([128, CB * nh], F32, tag="e0")
                        nc.vector.tensor_tensor(
                            out=e0[:].rearrange("p (b h) -> p b h", b=CB),
                            in0=gt[:, :, es_off:es_off + nh],
                            in1=p_ede[:].rearrange("p (b h) -> p b h", b=CB),
                            op=mybir.AluOpType.add)
                        e1 = wk.tile([128, CB * nh], F32, tag="e1")
                        nc.vector.tensor_scalar_mul(e1[:], e0[:], NEG)
                        e2 = wk.tile([128, CB * nh], F32, tag="e2")
                        nc.vector.tensor_tensor(out=e2[:], in0=e0[:], in1=e1[:],
                                                op=mybir.AluOpType.max)
                        exf = wk.tile([128, CB * nh], F32, tag="exf")
                        nc.scalar.activation(exf[:], e2[:],
                                             mybir.ActivationFunctionType.Exp)
                        ex = wk.tile([128, CB * nh], DT, tag="ex")
                        nc.vector.tensor_copy(ex[:], exf[:])
                        for j in range(CB):
                            b = c * CB + j
                            first, last = (b == 0), (b == NB - 1)
                            msg = wk.tile([128, hw], DT, tag="msg")
                            if l == 3:
                                nc.scalar.activation(
                                    msg[:, 0:ch], gt[:, j, 0:ch],
                                    mybir.ActivationFunctionType.Copy,
                                    scale=exf[:, j * nh:j * nh + 1])
                                if j % 2 == 0:
                                    nc.scalar.activation(
                                        msg[:, ch:2 * ch], gt[:, j, ch:2 * ch],
                                        mybir.ActivationFunctionType.Copy,
                                        scale=exf[:, j * nh + 1:j * nh + 2])
                                else:
                                    nc.vector.tensor_tensor(
                                        out=msg[:, ch:2 * ch],
                                        in0=gt[:, j, ch:2 * ch],
                                        in1=ex[:, j * nh + 1:j * nh + 2]
                                            .to_broadcast([128, ch]),
                                        op=mybir.AluOpType.mult)
                            else:
                                nc.scalar.activation(
                                    msg[:, 0:ch], gt[:, j, 0:ch],
                                    mybir.ActivationFunctionType.Copy,
                                    scale=exf[:, j * nh:j * nh + 1])
                                nc.vector.tensor_tensor(
                                    out=msg[:, ch:hw].rearrange(
                                        "p (h c) -> p h c", h=nh - 1),
                                    in0=gt[:, j, ch:hw].rearrange(
                                        "p (h c) -> p h c", h=nh - 1),
                                    in1=ex[:, j * nh + 1:(j + 1) * nh]
                                        .to_broadcast([128, nh - 1, ch]),
                                    op=mybir.AluOpType.mult)
                            nc.tensor.matmul(p_den[:],
                                             lhsT=sel[:, j, :],
                                             rhs=ex[:, j * nh:(j + 1) * nh],
                                             start=first, stop=last)
                            if l == 3:
                                nc.tensor.matmul(p_num0[:], lhsT=sel[:, j, :],
                                                 rhs=msg[:, 0:512],
                                                 start=first, stop=last)
                                nc.tensor.matmul(p_num1[:], lhsT=sel[:, j, :],
                                                 rhs=msg[:, 512:1024],
                                                 start=first, stop=last)
                            else:
                                nc.tensor.matmul(p_num[:], lhsT=sel[:, j, :],
                                                 rhs=msg[:],
                                                 start=first, stop=last)
                    # tile epilogue
                    dn = wk.tile([128, nh], F32, tag="dn", bufs=1)
                    nc.vector.tensor_scalar_add(dn[:], p_den[:], EPS)
                    rc = wk.tile([128, nh], F32, tag="rc", bufs=1)
                    nc.vector.reciprocal(rc[:], dn[:])
                    if l == 3:
                        a0 = wk.tile([128, 512], F32, tag="a0", bufs=1)
                        nc.vector.tensor_tensor(
                            out=a0[:], in0=p_num0[:],
                            in1=rc[:, 0:1].to_broadcast([128, 512]),
                            op=mybir.AluOpType.mult)
                        a1 = wk.tile([128, 512], F32, tag="a1", bufs=1)
                        nc.vector.tensor_tensor(
                            out=a1[:], in0=p_num1[:],
                            in1=rc[:, 1:2].to_broadcast([128, 512]),
                            op=mybir.AluOpType.mult)
                        s0 = wk.tile([128, 512], F32, tag="s0", bufs=1)
                        nc.vector.tensor_tensor(out=s0[:], in0=a0[:], in1=a1[:],
                                                op=mybir.AluOpType.add)
                        s1 = wk.tile([128, 512], F32, tag="s1", bufs=1)
                        nc.vector.tensor_scalar(
                            out=s1[:], in0=s0[:], scalar1=0.5, scalar2=None,
                            op0=mybir.AluOpType.mult)
                        s2 = wk.tile([128, 512], F32, tag="s2", bufs=1)
                        nc.vector.tensor_tensor(out=s2[:], in0=s1[:],
                                                in1=x_prev(t),
                                                op=mybir.AluOpType.add)
                        s3 = wk.tile([128, 512], F32, tag="s3", bufs=1)
                        nc.vector.tensor_tensor(out=s3[:], in0=s2[:],
                                                in1=b_bc[3][:],
                                                op=mybir.AluOpType.add)
                        nc.sync.dma_start(t_out[t * 128:(t + 1) * 128, :], s3[:])
                    else:
                        agg = wk.tile([128, 512], F32, tag="agg", bufs=1)
                        nc.vector.tensor_tensor(
                            out=agg[:].rearrange("p (h c) -> p h c", h=nh),
                            in0=p_num[:].rearrange("p (h c) -> p h c", h=nh),
                            in1=rc[:].to_broadcast([128, nh, ch]),
                            op=mybir.AluOpType.mult)
                        s0 = wk.tile([128, 512], F32, tag="s0", bufs=1)
                        nc.vector.tensor_tensor(out=s0[:], in0=agg[:],
                                                in1=x_prev(t),
                                                op=mybir.AluOpType.add)
                        s1 = wk.tile([128, 512], F32, tag="s1", bufs=1)
                        nc.vector.tensor_tensor(out=s1[:], in0=s0[:],
                                                in1=b_bc[l][:],
                                                op=mybir.AluOpType.add)
                        # elu(x) = max(x,0) + exp(min(x,0)) - 1
                        mn = wk.tile([128, 512], F32, tag="mn", bufs=1)
                        nc.vector.tensor_scalar_min(mn[:], s1[:], 0.0)
                        ep = wk.tile([128, 512], F32, tag="ep", bufs=1)
                        nc.scalar.activation(ep[:], mn[:],
                                             mybir.ActivationFunctionType.Exp)
                        mx = wk.tile([128, 512], F32, tag="mx", bufs=1)
                        nc.vector.tensor_scalar(out=mx[:], in0=s1[:], scalar1=0.0,
                                                scalar2=-1.0,
                                                op0=mybir.AluOpType.max,
                                                op1=mybir.AluOpType.add)
                        nc.vector.tensor_tensor(out=x_next[:, t, :], in0=mx[:],
                                                in1=ep[:],
                                                op=mybir.AluOpType.add)

            # ---- layer 1 ---------------------------------------------------
            tbl1_in = dram.tile([NPC, WT12], DT, tag="t1in")
            tbl1 = dram.tile([NPAD, WT12], DT, tag="t1", addr_space="Shared")

            def lhsT_l1(nt):
                x_nt = wk.tile([128, 4, 128], DT, tag="xnt")
                nc.sync.dma_start(
                    x_nt[:],
                    t_xgT[:].rearrange("(kb p) n -> p kb n", p=128)
                        [:, :, nt * 128:(nt + 1) * 128])
                return x_nt

            build_table(1, tbl1_in, lhsT_l1)
            nc.gpsimd.collective_compute(
                "AllGather", mybir.AluOpType.bypass, replica_groups=rg,
                ins=[tbl1_in[:]], outs=[tbl1[:]])

            def xprev1(t):
                xp = wk.tile([128, 512], F32, tag="xp1", bufs=1)
                nc.sync.dma_start(xp[:], t_xl[t * 128:(t + 1) * 128, :])
                return xp[:]

            edge_phase(1, tbl1, tbl1_in, xprev1, xA)

            # ---- layers 2, 3 ----------------------------------------------
            def transpose_lhsT(x_buf):
                def get(nt):
                    xt = wk.tile([128, 4, 128], DT, tag="xnt")
                    for kb in range(4):
                        p_t = psml.tile([128, 128], F32, space="PSUM", tag="small")
                        nc.tensor.transpose(
                            out=p_t[:],
                            in_=x_buf[:, nt, kb * 128:(kb + 1) * 128],
                            identity=id_f32[:])
                        nc.scalar.activation(xt[:, kb, :], p_t[:],
                                             mybir.ActivationFunctionType.Copy)
                    return xt
                return get

            tbl2_in = dram.tile([NPC, WT12], DT, tag="t2in")
            tbl2 = dram.tile([NPAD, WT12], DT, tag="t2", addr_space="Shared")
            build_table(2, tbl2_in, transpose_lhsT(xA))
            nc.gpsimd.collective_compute(
                "AllGather", mybir.AluOpType.bypass, replica_groups=rg,
                ins=[tbl2_in[:]], outs=[tbl2[:]])
            edge_phase(2, tbl2, tbl2_in, lambda t: xA[:, t, :], xB)

            tbl3_in = dram.tile([NPC, WT3], DT, tag="t3in")
            tbl3 = dram.tile([NPAD, WT3], DT, tag="t3", addr_space="Shared")
            build_table(3, tbl3_in, transpose_lhsT(xB))
            nc.gpsimd.collective_compute(
                "AllGather", mybir.AluOpType.bypass, replica_groups=rg,
                ins=[tbl3_in[:]], outs=[tbl3[:]])
            edge_phase(3, tbl3, tbl3_in, lambda t: xB[:, t, :], None)

    nc.compile()
    return nc


def _run(inputs, trace=False):
    in_maps, NB = _prep_host(**inputs)
    key = NB
    if key not in _cache:
        _cache[key] = _build_nc(NB)
    nc = _cache[key]
    res = run_bass_kernel_spmd(nc, in_maps, core_ids=list(range(NCORES)),
                               trace=trace)
    out = np.concatenate([r["out"] for r in res.results], axis=0)[:N]
    return out, res


def kernel(**inputs):
    out, _ = _run(inputs, trace=False)
    return out


# revision 11
# speedup vs baseline: 1.1622x; 1.0032x over previous
"""3-layer GAT (graph attention network) on 8 Trainium2 NeuronCores.

Strategy: node-sharded graph parallelism.
- Nodes padded 10000 -> 10240, 1280 per core; edges partitioned by dst range.
- Per layer: each core computes table rows [h | es | ed] for its nodes with a
  PE matmul against W_ext = [W | W@Bsrc | W@Bdst] (bf16), AllGathers the full
  table, then processes its edges: dma_gather of h[src] rows, attention
  softmax without segment-max (exponents are bounded, softmax is shift
  invariant), and scatter-by-matmul: sel[e,dst] one-hot matrices contract
  128-edge blocks into per-dst-tile PSUM accumulators for both the
  numerator (sum alpha*h) and denominator (sum exp e).
"""

import numpy as np
import ml_dtypes

import concourse.bass as bass
import concourse.bacc as bacc
import concourse.mybir as mybir
import concourse.tile as tile
from concourse.library_config import mlp
from concourse.masks import make_identity
from concourse.bass_utils import run_bass_kernel_spmd
from concourse._compat import cdiv

F32 = mybir.dt.float32
DT = mybir.dt.bfloat16
NPDT = ml_dtypes.bfloat16

N, E, D = 10000, 160000, 512
H, C = 4, 128
HF, CF = 2, 512
NEG = 0.2
EPS = 1e-16

NCORES = 8
NPAD = 10240
NPC = NPAD // NCORES       # 1280 nodes per core
NTL = NPC // 128           # 10 local dst tiles per core
NTG = NPAD // 128          # 80 global node tiles
CB = 6                     # gather-chunk size in 128-edge blocks (768 idxs)
WT12 = 640                 # bf16 table row width, layers 1-2 (512+4+4 -> pad)
WT3 = 1152                 # layer 3 (1024+2+2 -> pad)

_cache = {}


def _block_diag(a):
    """[Hh, Cc] attention vector -> [Hh*Cc, Hh] block-diagonal embed."""
    Hh, Cc = a.shape
    B = np.zeros((Hh * Cc, Hh), np.float32)
    for h in range(Hh):
        B[h * Cc:(h + 1) * Cc, h] = a[h]
    return B


def _prep_host(graph, edge_index, W1, as1, ad1, b1, W2, as2, ad2, b2,
               W3, as3, ad3, b3):
    src = np.asarray(edge_index[0], np.int64)
    dst = np.asarray(edge_index[1], np.int64)

    dstt = dst // 128
    cnt = np.bincount(dstt, minlength=NTG)
    nb = int(np.ceil(cnt.max() / 128))
    NB = max(18, int(cdiv(nb, CB)) * CB)          # blocks per dst tile
    NCHUNK = NB // CB
    order = np.argsort(dstt, kind="stable")
    off = np.concatenate([[0], np.cumsum(cnt)])

    idx_slots = np.zeros((NTG, NB * 128), np.int16)
    dstl_slots = np.full((NTG, NB * 128), 255.0, np.float32)
    for gt in range(NTG):
        e = order[off[gt]:off[gt + 1]]
        k = len(e)
        idx_slots[gt, :k] = src[e].astype(np.int16)
        dstl_slots[gt, :k] = (dst[e] - gt * 128).astype(np.float32)

    # dma_gather wrapped index layout: within each 768-idx chunk,
    # unwrapped[j] = wrapped[j % 16, j // 16]; replicated to 128 partitions.
    w = idx_slots.reshape(NTG, NCHUNK, 48, 16).transpose(0, 1, 3, 2)  # [.,.,16,48]
    w = np.tile(w, (1, 1, 8, 1))                                      # [.,.,128,48]
    # dstl column layout: [p, b] = slot b*128+p
    dcol = dstl_slots.reshape(NTG, NB, 128).transpose(0, 2, 1)        # [NTG,128,NB]

    xpad = np.zeros((NPAD, D), np.float32)
    xpad[:N] = np.asarray(graph, np.float32)

    def wext(W, a_s, a_d, wt):
        cols = np.concatenate(
            [W, W @ _block_diag(a_s), W @ _block_diag(a_d)], axis=1)
        out = np.zeros((D, wt), np.float32)
        out[:, :cols.shape[1]] = cols
        return out.astype(NPDT)

    we1 = wext(np.asarray(W1, np.float32), np.asarray(as1), np.asarray(ad1), WT12)
    we2 = wext(np.asarray(W2, np.float32), np.asarray(as2), np.asarray(ad2), WT12)
    we3 = wext(np.asarray(W3, np.float32), np.asarray(as3), np.asarray(ad3), WT3)

    iotaF6 = np.tile(np.arange(128, dtype=np.float32)[None, :], (128, 6)).astype(NPDT)
    iotaP = np.arange(128, dtype=np.float32)[:, None].astype(NPDT)
    ones_row = np.ones((1, 128), np.float32)

    in_maps = []
    for c in range(NCORES):
        tl = slice(c * NTL, (c + 1) * NTL)
        idx_c = w[tl].transpose(2, 0, 1, 3).reshape(128, NTL * NCHUNK * 48)
        dstl_c = dcol[tl].transpose(1, 0, 2).reshape(128, NTL * NB).astype(NPDT)
        dstlb_c = np.tile(dstl_slots[tl].reshape(1, NTL * NB * 128),
                          (128, 1)).astype(NPDT)
        xgT_c = np.ascontiguousarray(
            xpad[c * NPC:(c + 1) * NPC].T).astype(NPDT)   # [512, 1280]
        xl_c = xpad[c * NPC:(c + 1) * NPC]                 # [1280, 512] f32
        in_maps.append({
            "idx": np.ascontiguousarray(idx_c),
            "dstl": np.ascontiguousarray(dstl_c),
            "dstlb": np.ascontiguousarray(dstlb_c),
            "xgT": xgT_c,
            "xl": np.ascontiguousarray(xl_c),
            "we1": we1, "we2": we2, "we3": we3,
            "b1": np.asarray(b1, np.float32)[None, :],
            "b2": np.asarray(b2, np.float32)[None, :],
            "b3": np.asarray(b3, np.float32)[None, :],
            "iotaF6": iotaF6,
            "iotaP": iotaP,
            "ones": ones_row,
        })
    return in_maps, NB


def _build_nc(NB):
    NCHUNK = NB // CB
    nc = bacc.Bacc("TRN2", target_bir_lowering=False, debug=False,
                   num_devices=NCORES, num_swdge_queues=4)

    t_idx = nc.dram_tensor("idx", [128, NTL * NCHUNK * 48], mybir.dt.int16,
                           kind="ExternalInput")
    t_dstl = nc.dram_tensor("dstl", [128, NTL * NB], DT, kind="ExternalInput")
    t_dstlb = nc.dram_tensor("dstlb", [128, NTL * NB * 128], DT,
                             kind="ExternalInput")
    t_xgT = nc.dram_tensor("xgT", [D, NPC], DT, kind="ExternalInput")
    t_xl = nc.dram_tensor("xl", [NPC, D], F32, kind="ExternalInput")
    t_we = {1: nc.dram_tensor("we1", [D, WT12], DT, kind="ExternalInput"),
            2: nc.dram_tensor("we2", [D, WT12], DT, kind="ExternalInput"),
            3: nc.dram_tensor("we3", [D, WT3], DT, kind="ExternalInput")}
    t_b = {1: nc.dram_tensor("b1", [1, D], F32, kind="ExternalInput"),
           2: nc.dram_tensor("b2", [1, D], F32, kind="ExternalInput"),
           3: nc.dram_tensor("b3", [1, D], F32, kind="ExternalInput")}
    t_iotaF6 = nc.dram_tensor("iotaF6", [128, 768], DT, kind="ExternalInput")
    t_iotaP = nc.dram_tensor("iotaP", [128, 1], DT, kind="ExternalInput")
    t_ones = nc.dram_tensor("ones", [1, 128], F32, kind="ExternalInput")
    t_out = nc.dram_tensor("out", [NPC, D], F32, kind="ExternalOutput")

    rg = [list(range(NCORES))]
    qn = [0]

    with tile.TileContext(nc) as tc:
        with tc.tile_pool(name="cst", bufs=1) as cst, \
             tc.tile_pool(name="per", bufs=1) as per, \
             tc.tile_pool(name="wk", bufs=2) as wk, \
             tc.tile_pool(name="ed", bufs=2) as edp, \
             tc.tile_pool(name="gath", bufs=4) as gp, \
             tc.tile_pool(name="pnum", bufs=2, space="PSUM") as pnum, \
             tc.tile_pool(name="pden", bufs=2, space="PSUM") as pden, \
             tc.tile_pool(name="psmall", bufs=2, space="PSUM") as psml, \
             tc.tile_pool(name="dram", bufs=1, space="DRAM") as dram:

            nc.gpsimd.load_library(mlp)

            # ---- constants -------------------------------------------------
            iotaF6 = cst.tile([128, 768], DT)
            nc.sync.dma_start(iotaF6[:], t_iotaF6[:])
            iotaP = cst.tile([128, 1], DT)
            nc.sync.dma_start(iotaP[:], t_iotaP[:])
            id_f32 = cst.tile([128, 128], F32)
            make_identity(nc, id_f32[:])
            ones_t = cst.tile([1, 128], F32)
            nc.sync.dma_start(ones_t[:], t_ones[:])

            idx_t = per.tile([128, NTL * NCHUNK * 48], mybir.dt.int16)
            nc.sync.dma_start(idx_t[:], t_idx[:])
            dstl = per.tile([128, NTL * NB], DT)
            nc.sync.dma_start(dstl[:], t_dstl[:])

            b_bc = {}
            for l in (1, 2, 3):
                br = wk.tile([1, D], F32, tag="brow", bufs=1)
                nc.sync.dma_start(br[:], t_b[l][:])
                pb = pnum.tile([128, D], F32, space="PSUM", tag="numA")
                nc.tensor.matmul(pb[:], lhsT=ones_t[:], rhs=br[:],
                                 start=True, stop=True)
                b_bc[l] = cst.tile([128, D], F32, tag=f"bbc{l}", name=f"bbc{l}")
                nc.vector.tensor_copy(b_bc[l][:], pb[:])

            we = {}
            for l in (1, 2, 3):
                wt = WT3 if l == 3 else WT12
                we[l] = per.tile([128, 4, wt], DT, tag=f"we{l}", name=f"we{l}")
                nc.sync.dma_start(
                    we[l][:],
                    t_we[l][:].rearrange("(kb p) w -> p kb w", p=128))

            # persistent x buffers (f32) for layer 2/3 inputs
            xA = per.tile([128, NTL, D], F32, tag="xA")
            xB = per.tile([128, NTL, D], F32, tag="xB")

            # ---- per-layer helpers ----------------------------------------
            def build_table(l, tbl_in, get_lhsT):
                """Local table rows: [h | es | ed] for this core's nodes."""
                wt = WT3 if l == 3 else WT12
                hw = HF * CF if l == 3 else H * C
                nh = HF if l == 3 else H
                segs = [(0, 512), (512, 1024), (1024, 1024 + 2 * nh)] if l == 3 \
                    else [(0, 512), (512, 512 + 2 * nh)]
                for nt in range(NTL):
                    lhsT = get_lhsT(nt)  # [128, 4, 128] DT tile
                    psums = []
                    for si, (c0, c1) in enumerate(segs):
                        if c1 - c0 > 64:
                            pool_, tag = pnum, ("numA" if si == 0 else "numB")
                        else:
                            pool_, tag = psml, "small"
                        p = pool_.tile([128, c1 - c0], F32, space="PSUM",
                                       tag=tag, name=f"p_tbl{si}")
                        for kb in range(4):
                            nc.tensor.matmul(p[:], lhsT=lhsT[:, kb, :],
                                             rhs=we[l][:, kb, c0:c1],
                                             start=(kb == 0), stop=(kb == 3))
                        psums.append((c0, c1, p))
                    row = wk.tile([128, wt], DT, tag="tblrow")
                    for c0, c1, p in psums:
                        nc.vector.tensor_copy(row[:, c0:c1], p[:])
                    nc.sync.dma_start(tbl_in[nt * 128:(nt + 1) * 128, :], row[:])

            def edge_phase(l, tbl, tbl_in, x_prev, x_next):
                wt = WT3 if l == 3 else WT12
                nh = HF if l == 3 else H
                ch = CF if l == 3 else C
                hw = nh * ch
                es_off, ed_off = hw, hw + nh
                for t in range(NTL):
                    dstlb_t = wk.tile([128, NB * 128], DT, tag="dstlb")
                    nc.sync.dma_start(
                        dstlb_t[:],
                        t_dstlb[:, t * NB * 128:(t + 1) * NB * 128])
                    ed_t = edp.tile([128, nh], DT, tag="edt")
                    nc.sync.dma_start(
                        ed_t[:],
                        tbl_in[t * 128:(t + 1) * 128, ed_off:ed_off + nh])
                    if l == 3:
                        p_num0 = pnum.tile([128, 512], F32, space="PSUM", tag="numA")
                        p_num1 = pnum.tile([128, 512], F32, space="PSUM", tag="numB")
                    else:
                        p_num = pnum.tile([128, 512], F32, space="PSUM", tag="numA")
                    p_den = pden.tile([128, nh], F32, space="PSUM", tag="den")
                    for c in range(NB // CB):
                        gt = gp.tile([128, CB, wt], DT, tag="gt")
                        icol = (t * (NB // CB) + c) * 48
                        nc.gpsimd.dma_gather(
                            gt[:], tbl[:], idx_t[:, icol:icol + 48],
                            CB * 128, CB * 128, wt, queue_num=qn[0] % 4)
                        qn[0] += 1
                        scol = c * CB * 128
                        selT = wk.tile([128, CB * 128], DT, tag="selT")
                        nc.vector.tensor_tensor(
                            out=selT[:], in0=iotaP[:].to_broadcast([128, CB * 128]),
                            in1=dstlb_t[:, scol:scol + CB * 128],
                            op=mybir.AluOpType.is_equal)
                        sel = wk.tile([128, CB, 128], DT, tag="sel")
                        dc0 = t * NB + c * CB
                        nc.vector.tensor_tensor(
                            out=sel[:],
                            in0=dstl[:, dc0:dc0 + CB, None].to_broadcast(
                                [128, CB, 128]),
                            in1=iotaF6[:].rearrange("p (b f) -> p b f", b=CB),
                            op=mybir.AluOpType.is_equal)
                        p_ede = psml.tile([128, CB * nh], F32, space="PSUM",
                                          tag="small")
                        for j in range(CB):
                            nc.tensor.matmul(
                                p_ede[:, j * nh:(j + 1) * nh],
                                lhsT=selT[:, j * 128:(j + 1) * 128],
                                rhs=ed_t[:], start=True, stop=True)
                        e0 = wk.tile([128, CB * nh], F32, tag="e0")
                        nc.vector.tensor_tensor(
                            out=e0[:].rearrange("p (b h) -> p b h", b=CB),
                            in0=gt[:, :, es_off:es_off + nh],
                            in1=p_ede[:].rearrange("p (b h) -> p b h", b=CB),
                            op=mybir.AluOpType.add)
                        e1 = wk.tile([128, CB * nh], F32, tag="e1")
                        nc.vector.tensor_scalar_mul(e1[:], e0[:], NEG)
                        e2 = wk.tile([128, CB * nh], F32, tag="e2")
                        nc.vector.tensor_tensor(out=e2[:], in0=e0[:], in1=e1[:],
                                                op=mybir.AluOpType.max)
                        exf = wk.tile([128, CB * nh], F32, tag="exf")
                        nc.scalar.activation(exf[:], e2[:],
                                             mybir.ActivationFunctionType.Exp)
                        ex = wk.tile([128, CB * nh], DT, tag="ex")
                        nc.vector.tensor_copy(ex[:], exf[:])
                        for j in range(CB):
                            b = c * CB + j
                            first, last = (b == 0), (b == NB - 1)
                            msg = wk.tile([128, hw], DT, tag="msg")
                            if l == 3:
                                nc.scalar.activation(
                                    msg[:, 0:ch], gt[:, j, 0:ch],
                                    mybir.ActivationFunctionType.Copy,
                                    scale=exf[:, j * nh:j * nh + 1])
                                if j % 2 == 0:
                                    nc.scalar.activation(
                                        msg[:, ch:2 * ch], gt[:, j, ch:2 * ch],
                                        mybir.ActivationFunctionType.Copy,
                                        scale=exf[:, j * nh + 1:j * nh + 2])
                                else:
                                    nc.vector.tensor_tensor(
                                        out=msg[:, ch:2 * ch],
                                        in0=gt[:, j, ch:2 * ch],
                                        in1=ex[:, j * nh + 1:j * nh + 2]
                                            .to_broadcast([128, ch]),
                                        op=mybir.AluOpType.mult)
                            else:
                                for h in range(2):
                                    nc.scalar.activation(
                                        msg[:, h * ch:(h + 1) * ch],
                                        gt[:, j, h * ch:(h + 1) * ch],
                                        mybir.ActivationFunctionType.Copy,
                                        scale=exf[:, j * nh + h:j * nh + h + 1])
                                nc.vector.tensor_tensor(
                                    out=msg[:, 2 * ch:hw].rearrange(
                                        "p (h c) -> p h c", h=nh - 2),
                                    in0=gt[:, j, 2 * ch:hw].rearrange(
                                        "p (h c) -> p h c", h=nh - 2),
                                    in1=ex[:, j * nh + 2:(j + 1) * nh]
                                        .to_broadcast([128, nh - 2, ch]),
                                    op=mybir.AluOpType.mult)
                            nc.tensor.matmul(p_den[:],
                                             lhsT=sel[:, j, :],
                                             rhs=ex[:, j * nh:(j + 1) * nh],
                                             start=first, stop=last)
                            if l == 3:
                                nc.tensor.matmul(p_num0[:], lhsT=sel[:, j, :],
                                                 rhs=msg[:, 0:512],
                                                 start=first, stop=last)
                                nc.tensor.matmul(p_num1[:], lhsT=sel[:, j, :],
                                                 rhs=msg[:, 512:1024],
                                                 start=first, stop=last)
                            else:
                                nc.tensor.matmul(p_num[:], lhsT=sel[:, j, :],
                                                 rhs=msg[:],
                                                 start=first, stop=last)
                    # tile epilogue
                    dn = wk.tile([128, nh], F32, tag="dn", bufs=1)
                    nc.vector.tensor_scalar_add(dn[:], p_den[:], EPS)
                    rc = wk.tile([128, nh], F32, tag="rc", bufs=1)
                    nc.vector.reciprocal(rc[:], dn[:])
                    if l == 3:
                        a0 = wk.tile([128, 512], F32, tag="a0", bufs=1)
                        nc.vector.tensor_tensor(
                            out=a0[:], in0=p_num0[:],
                            in1=rc[:, 0:1].to_broadcast([128, 512]),
                            op=mybir.AluOpType.mult)
                        a1 = wk.tile([128, 512], F32, tag="a1", bufs=1)
                        nc.vector.tensor_tensor(
                            out=a1[:], in0=p_num1[:],
                            in1=rc[:, 1:2].to_broadcast([128, 512]),
                            op=mybir.AluOpType.mult)
                        s0 = wk.tile([128, 512], F32, tag="s0", bufs=1)
                        nc.vector.tensor_tensor(out=s0[:], in0=a0[:], in1=a1[:],
                                                op=mybir.AluOpType.add)
                        s1 = wk.tile([128, 512], F32, tag="s1", bufs=1)
                        nc.vector.tensor_scalar(
                            out=s1[:], in0=s0[:], scalar1=0.5, scalar2=None,
                            op0=mybir.AluOpType.mult)
                        s2 = wk.tile([128, 512], F32, tag="s2", bufs=1)
                        nc.vector.tensor_tensor(out=s2[:], in0=s1[:],
                                                in1=x_prev(t),
                                                op=mybir.AluOpType.add)
                        s3 = wk.tile([128, 512], F32, tag="s3", bufs=1)
                        nc.vector.tensor_tensor(out=s3[:], in0=s2[:],
                                                in1=b_bc[3][:],
                                                op=mybir.AluOpType.add)
                        nc.sync.dma_start(t_out[t * 128:(t + 1) * 128, :], s3[:])
                    else:
                        agg = wk.tile([128, 512], F32, tag="agg", bufs=1)
                        nc.vector.tensor_tensor(
                            out=agg[:].rearrange("p (h c) -> p h c", h=nh),
                            in0=p_num[:].rearrange("p (h c) -> p h c", h=nh),
                            in1=rc[:].to_broadcast([128, nh, ch]),
                            op=mybir.AluOpType.mult)
                        s0 = wk.tile([128, 512], F32, tag="s0", bufs=1)
                        nc.vector.tensor_tensor(out=s0[:], in0=agg[:],
                                                in1=x_prev(t),
                                                op=mybir.AluOpType.add)
                        s1 = wk.tile([128, 512], F32, tag="s1", bufs=1)
                        nc.vector.tensor_tensor(out=s1[:], in0=s0[:],
                                                in1=b_bc[l][:],
                                                op=mybir.AluOpType.add)
                        # elu(x) = max(x,0) + exp(min(x,0)) - 1
                        mn = wk.tile([128, 512], F32, tag="mn", bufs=1)
                        nc.vector.tensor_scalar_min(mn[:], s1[:], 0.0)
                        ep = wk.tile([128, 512], F32, tag="ep", bufs=1)
                        nc.scalar.activation(ep[:], mn[:],
                                             mybir.ActivationFunctionType.Exp)
                        mx = wk.tile([128, 512], F32, tag="mx", bufs=1)
                        nc.vector.tensor_scalar(out=mx[:], in0=s1[:], scalar1=0.0,
                                                scalar2=-1.0,
                                                op0=mybir.AluOpType.max,
                                                op1=mybir.AluOpType.add)
                        nc.vector.tensor_tensor(out=x_next[:, t, :], in0=mx[:],
                                                in1=ep[:],
                                                op=mybir.AluOpType.add)

            # ---- layer 1 ---------------------------------------------------
            tbl1_in = dram.tile([NPC, WT12], DT, tag="t1in")
            tbl1 = dram.tile([NPAD, WT12], DT, tag="t1", addr_space="Shared")

            def lhsT_l1(nt):
                x_nt = wk.tile([128, 4, 128], DT, tag="xnt")
                nc.sync.dma_start(
                    x_nt[:],
                    t_xgT[:].rearrange("(kb p) n -> p kb n", p=128)
                        [:, :, nt * 128:(nt + 1) * 128])
                return x_nt

            build_table(1, tbl1_in, lhsT_l1)
            nc.gpsimd.collective_compute(
                "AllGather", mybir.AluOpType.bypass, replica_groups=rg,
                ins=[tbl1_in[:]], outs=[tbl1[:]])

            def xprev1(t):
                xp = wk.tile([128, 512], F32, tag="xp1", bufs=1)
                nc.sync.dma_start(xp[:], t_xl[t * 128:(t + 1) * 128, :])
                return xp[:]

            edge_phase(1, tbl1, tbl1_in, xprev1, xA)

            # ---- layers 2, 3 ----------------------------------------------
            def transpose_lhsT(x_buf):
                def get(nt):
                    xt = wk.tile([128, 4, 128], DT, tag="xnt")
                    for kb in range(4):
                        p_t = psml.tile([128, 128], F32, space="PSUM", tag="small")
                        nc.tensor.transpose(
                            out=p_t[:],
                            in_=x_buf[:, nt, kb * 128:(kb + 1) * 128],
                            identity=id_f32[:])
                        nc.scalar.activation(xt[:, kb, :], p_t[:],
                                             mybir.ActivationFunctionType.Copy)
                    return xt
                return get

            tbl2_in = dram.tile([NPC, WT12], DT, tag="t2in")
            tbl2 = dram.tile([NPAD, WT12], DT, tag="t2", addr_space="Shared")
            build_table(2, tbl2_in, transpose_lhsT(xA))
            nc.gpsimd.collective_compute(
                "AllGather", mybir.AluOpType.bypass, replica_groups=rg,
                ins=[tbl2_in[:]], outs=[tbl2[:]])
            edge_phase(2, tbl2, tbl2_in, lambda t: xA[:, t, :], xB)

            tbl3_in = dram.tile([NPC, WT3], DT, tag="t3in")
            tbl3 = dram.tile([NPAD, WT3], DT, tag="t3", addr_space="Shared")
            build_table(3, tbl3_in, transpose_lhsT(xB))
            nc.gpsimd.collective_compute(
                "AllGather", mybir.AluOpType.bypass, replica_groups=rg,
                ins=[tbl3_in[:]], outs=[tbl3[:]])
            edge_phase(3, tbl3, tbl3_in, lambda t: xB[:, t, :], None)

    nc.compile()
    return nc


def _run(inputs, trace=False):
    in_maps, NB = _prep_host(**inputs)
    key = NB
    if key not in _cache:
        _cache[key] = _build_nc(NB)
    nc = _cache[key]
    res = run_bass_kernel_spmd(nc, in_maps, core_ids=list(range(NCORES)),
                               trace=trace)
    out = np.concatenate([r["out"] for r in res.results], axis=0)[:N]
    return out, res


def kernel(**inputs):
    out, _ = _run(inputs, trace=False)
    return out


# revision 12
# speedup vs baseline: 1.1810x; 1.0162x over previous
"""3-layer GAT (graph attention network) on 8 Trainium2 NeuronCores.

Strategy: node-sharded graph parallelism.
- Nodes padded 10000 -> 10240, 1280 per core; edges partitioned by dst range.
- Per layer: each core computes table rows [h | es | ed] for its nodes with a
  PE matmul against W_ext = [W | W@Bsrc | W@Bdst] (bf16), AllGathers the full
  table, then processes its edges: dma_gather of h[src] rows, attention
  softmax without segment-max (exponents are bounded, softmax is shift
  invariant), and scatter-by-matmul: sel[e,dst] one-hot matrices contract
  128-edge blocks into per-dst-tile PSUM accumulators for both the
  numerator (sum alpha*h) and denominator (sum exp e).
"""

import numpy as np
import ml_dtypes

import concourse.bass as bass
import concourse.bacc as bacc
import concourse.mybir as mybir
import concourse.tile as tile
from concourse.library_config import mlp
from concourse.masks import make_identity
from concourse.bass_utils import run_bass_kernel_spmd
from concourse._compat import cdiv

F32 = mybir.dt.float32
DT = mybir.dt.bfloat16
NPDT = ml_dtypes.bfloat16

N, E, D = 10000, 160000, 512
H, C = 4, 128
HF, CF = 2, 512
NEG = 0.2
EPS = 1e-16

NCORES = 8
NPAD = 10240
NPC = NPAD // NCORES       # 1280 nodes per core
NTL = NPC // 128           # 10 local dst tiles per core
NTG = NPAD // 128          # 80 global node tiles
CB = 6                     # gather-chunk size in 128-edge blocks (768 idxs)
WT12 = 640                 # bf16 table row width, layers 1-2 (512+4+4 -> pad)
WT3 = 1152                 # layer 3 (1024+2+2 -> pad)

_cache = {}


def _block_diag(a):
    """[Hh, Cc] attention vector -> [Hh*Cc, Hh] block-diagonal embed."""
    Hh, Cc = a.shape
    B = np.zeros((Hh * Cc, Hh), np.float32)
    for h in range(Hh):
        B[h * Cc:(h + 1) * Cc, h] = a[h]
    return B


def _prep_host(graph, edge_index, W1, as1, ad1, b1, W2, as2, ad2, b2,
               W3, as3, ad3, b3):
    src = np.asarray(edge_index[0], np.int64)
    dst = np.asarray(edge_index[1], np.int64)

    dstt = dst // 128
    cnt = np.bincount(dstt, minlength=NTG)
    nb = int(np.ceil(cnt.max() / 128))
    NB = max(18, int(cdiv(nb, CB)) * CB)          # blocks per dst tile
    NCHUNK = NB // CB
    order = np.argsort(dstt, kind="stable")
    off = np.concatenate([[0], np.cumsum(cnt)])

    idx_slots = np.zeros((NTG, NB * 128), np.int16)
    dstl_slots = np.full((NTG, NB * 128), 255.0, np.float32)
    for gt in range(NTG):
        e = order[off[gt]:off[gt + 1]]
        k = len(e)
        idx_slots[gt, :k] = src[e].astype(np.int16)
        dstl_slots[gt, :k] = (dst[e] - gt * 128).astype(np.float32)

    # dma_gather wrapped index layout: within each 768-idx chunk,
    # unwrapped[j] = wrapped[j % 16, j // 16]; replicated to 128 partitions.
    w = idx_slots.reshape(NTG, NCHUNK, 48, 16).transpose(0, 1, 3, 2)  # [.,.,16,48]
    w = np.tile(w, (1, 1, 8, 1))                                      # [.,.,128,48]
    # selection matrices, shared across layers (depend only on edge layout):
    # sel[p, (t,c,j,d)]: slot (c*6+j)*128+p of tile t targets local dst d
    # selT[d, (t,c,j,e)]: transpose
    oh = (dstl_slots[:, :, None] ==
          np.arange(128, dtype=np.float32)[None, None, :])            # [NTG,NB*128,128]
    oh = oh.reshape(NTG, NCHUNK, CB, 128, 128)                        # [t,c,j,p,d]
    sel_h = oh.transpose(0, 1, 3, 2, 4).astype(NPDT)                  # [t,c,p,j,d]
    selT_h = oh.transpose(0, 1, 4, 2, 3).astype(NPDT)                 # [t,c,d,j,e]

    xpad = np.zeros((NPAD, D), np.float32)
    xpad[:N] = np.asarray(graph, np.float32)

    def wext(W, a_s, a_d, wt):
        cols = np.concatenate(
            [W, W @ _block_diag(a_s), W @ _block_diag(a_d)], axis=1)
        out = np.zeros((D, wt), np.float32)
        out[:, :cols.shape[1]] = cols
        return out.astype(NPDT)

    we1 = wext(np.asarray(W1, np.float32), np.asarray(as1), np.asarray(ad1), WT12)
    we2 = wext(np.asarray(W2, np.float32), np.asarray(as2), np.asarray(ad2), WT12)
    we3 = wext(np.asarray(W3, np.float32), np.asarray(as3), np.asarray(ad3), WT3)

    ones_row = np.ones((1, 128), np.float32)

    in_maps = []
    for c in range(NCORES):
        tl = slice(c * NTL, (c + 1) * NTL)
        idx_c = w[tl].transpose(2, 0, 1, 3).reshape(128, NTL * NCHUNK * 48)
        sel_c = sel_h[tl].transpose(2, 0, 1, 3, 4).reshape(128, NTL * NCHUNK * 768)
        selT_c = selT_h[tl].transpose(2, 0, 1, 3, 4).reshape(128, NTL * NCHUNK * 768)
        xgT_c = np.ascontiguousarray(
            xpad[c * NPC:(c + 1) * NPC].T).astype(NPDT)   # [512, 1280]
        xl_c = xpad[c * NPC:(c + 1) * NPC]                 # [1280, 512] f32
        in_maps.append({
            "idx": np.ascontiguousarray(idx_c),
            "selh": np.ascontiguousarray(sel_c),
            "selTh": np.ascontiguousarray(selT_c),
            "xgT": xgT_c,
            "xl": np.ascontiguousarray(xl_c),
            "we1": we1, "we2": we2, "we3": we3,
            "b1": np.asarray(b1, np.float32)[None, :],
            "b2": np.asarray(b2, np.float32)[None, :],
            "b3": np.asarray(b3, np.float32)[None, :],
            "ones": ones_row,
        })
    return in_maps, NB


def _build_nc(NB):
    NCHUNK = NB // CB
    nc = bacc.Bacc("TRN2", target_bir_lowering=False, debug=False,
                   num_devices=NCORES, num_swdge_queues=4)

    t_idx = nc.dram_tensor("idx", [128, NTL * NCHUNK * 48], mybir.dt.int16,
                           kind="ExternalInput")
    t_selh = nc.dram_tensor("selh", [128, NTL * NCHUNK * 768], DT,
                            kind="ExternalInput")
    t_selTh = nc.dram_tensor("selTh", [128, NTL * NCHUNK * 768], DT,
                             kind="ExternalInput")
    t_xgT = nc.dram_tensor("xgT", [D, NPC], DT, kind="ExternalInput")
    t_xl = nc.dram_tensor("xl", [NPC, D], F32, kind="ExternalInput")
    t_we = {1: nc.dram_tensor("we1", [D, WT12], DT, kind="ExternalInput"),
            2: nc.dram_tensor("we2", [D, WT12], DT, kind="ExternalInput"),
            3: nc.dram_tensor("we3", [D, WT3], DT, kind="ExternalInput")}
    t_b = {1: nc.dram_tensor("b1", [1, D], F32, kind="ExternalInput"),
           2: nc.dram_tensor("b2", [1, D], F32, kind="ExternalInput"),
           3: nc.dram_tensor("b3", [1, D], F32, kind="ExternalInput")}

    t_ones = nc.dram_tensor("ones", [1, 128], F32, kind="ExternalInput")
    t_out = nc.dram_tensor("out", [NPC, D], F32, kind="ExternalOutput")

    rg = [list(range(NCORES))]
    qn = [0]

    with tile.TileContext(nc) as tc:
        with tc.tile_pool(name="cst", bufs=1) as cst, \
             tc.tile_pool(name="per", bufs=1) as per, \
             tc.tile_pool(name="wk", bufs=2) as wk, \
             tc.tile_pool(name="ed", bufs=2) as edp, \
             tc.tile_pool(name="gath", bufs=4) as gp, \
             tc.tile_pool(name="pnum", bufs=2, space="PSUM") as pnum, \
             tc.tile_pool(name="pden", bufs=2, space="PSUM") as pden, \
             tc.tile_pool(name="psmall", bufs=2, space="PSUM") as psml, \
             tc.tile_pool(name="dram", bufs=1, space="DRAM") as dram:

            nc.gpsimd.load_library(mlp)

            # ---- constants -------------------------------------------------
            id_f32 = cst.tile([128, 128], F32)
            make_identity(nc, id_f32[:])
            ones_t = cst.tile([1, 128], F32)
            nc.sync.dma_start(ones_t[:], t_ones[:])

            idx_t = per.tile([128, NTL * NCHUNK * 48], mybir.dt.int16)
            nc.sync.dma_start(idx_t[:], t_idx[:])

            b_bc = {}
            for l in (1, 2, 3):
                br = wk.tile([1, D], F32, tag="brow", bufs=1)
                nc.sync.dma_start(br[:], t_b[l][:])
                pb = pnum.tile([128, D], F32, space="PSUM", tag="numA")
                nc.tensor.matmul(pb[:], lhsT=ones_t[:], rhs=br[:],
                                 start=True, stop=True)
                b_bc[l] = cst.tile([128, D], F32, tag=f"bbc{l}", name=f"bbc{l}")
                nc.vector.tensor_copy(b_bc[l][:], pb[:])

            we = {}
            for l in (1, 2, 3):
                wt = WT3 if l == 3 else WT12
                we[l] = per.tile([128, 4, wt], DT, tag=f"we{l}", name=f"we{l}")
                nc.sync.dma_start(
                    we[l][:],
                    t_we[l][:].rearrange("(kb p) w -> p kb w", p=128))

            # persistent x buffers (f32) for layer 2/3 inputs
            xA = per.tile([128, NTL, D], F32, tag="xA")
            xB = per.tile([128, NTL, D], F32, tag="xB")

            # ---- per-layer helpers ----------------------------------------
            def build_table(l, tbl_in, get_lhsT):
                """Local table rows: [h | es | ed] for this core's nodes."""
                wt = WT3 if l == 3 else WT12
                hw = HF * CF if l == 3 else H * C
                nh = HF if l == 3 else H
                segs = [(0, 512), (512, 1024), (1024, 1024 + 2 * nh)] if l == 3 \
                    else [(0, 512), (512, 512 + 2 * nh)]
                for nt in range(NTL):
                    lhsT = get_lhsT(nt)  # [128, 4, 128] DT tile
                    psums = []
                    for si, (c0, c1) in enumerate(segs):
                        if c1 - c0 > 64:
                            pool_, tag = pnum, ("numA" if si == 0 else "numB")
                        else:
                            pool_, tag = psml, "small"
                        p = pool_.tile([128, c1 - c0], F32, space="PSUM",
                                       tag=tag, name=f"p_tbl{si}")
                        for kb in range(4):
                            nc.tensor.matmul(p[:], lhsT=lhsT[:, kb, :],
                                             rhs=we[l][:, kb, c0:c1],
                                             start=(kb == 0), stop=(kb == 3))
                        psums.append((c0, c1, p))
                    row = wk.tile([128, wt], DT, tag="tblrow")
                    for c0, c1, p in psums:
                        nc.vector.tensor_copy(row[:, c0:c1], p[:])
                    nc.sync.dma_start(tbl_in[nt * 128:(nt + 1) * 128, :], row[:])

            def edge_phase(l, tbl, tbl_in, x_prev, x_next):
                wt = WT3 if l == 3 else WT12
                nh = HF if l == 3 else H
                ch = CF if l == 3 else C
                hw = nh * ch
                es_off, ed_off = hw, hw + nh
                for t in range(NTL):
                    ed_t = edp.tile([128, nh], DT, tag="edt")
                    nc.sync.dma_start(
                        ed_t[:],
                        tbl_in[t * 128:(t + 1) * 128, ed_off:ed_off + nh])
                    if l == 3:
                        p_num0 = pnum.tile([128, 512], F32, space="PSUM", tag="numA")
                        p_num1 = pnum.tile([128, 512], F32, space="PSUM", tag="numB")
                    else:
                        p_num = pnum.tile([128, 512], F32, space="PSUM", tag="numA")
                    p_den = pden.tile([128, nh], F32, space="PSUM", tag="den")
                    for c in range(NB // CB):
                        gt = gp.tile([128, CB, wt], DT, tag="gt")
                        icol = (t * (NB // CB) + c) * 48
                        nc.gpsimd.dma_gather(
                            gt[:], tbl[:], idx_t[:, icol:icol + 48],
                            CB * 128, CB * 128, wt, queue_num=qn[0] % 4)
                        qn[0] += 1
                        scol = (t * NCHUNK + c) * 768
                        selT = wk.tile([128, CB * 128], DT, tag="selT", bufs=3)
                        nc.sync.dma_start(selT[:], t_selTh[:, scol:scol + 768])
                        sel = wk.tile([128, CB, 128], DT, tag="sel", bufs=3)
                        nc.sync.dma_start(
                            sel[:], t_selh[:, scol:scol + 768]
                            .rearrange("p (b f) -> p b f", b=CB))
                        p_ede = psml.tile([128, CB * nh], F32, space="PSUM",
                                          tag="small")
                        for j in range(CB):
                            nc.tensor.matmul(
                                p_ede[:, j * nh:(j + 1) * nh],
                                lhsT=selT[:, j * 128:(j + 1) * 128],
                                rhs=ed_t[:], start=True, stop=True)
                        e0 = wk.tile([128, CB * nh], F32, tag="e0")
                        nc.vector.tensor_tensor(
                            out=e0[:].rearrange("p (b h) -> p b h", b=CB),
                            in0=gt[:, :, es_off:es_off + nh],
                            in1=p_ede[:].rearrange("p (b h) -> p b h", b=CB),
                            op=mybir.AluOpType.add)
                        e1 = wk.tile([128, CB * nh], F32, tag="e1")
                        nc.vector.tensor_scalar_mul(e1[:], e0[:], NEG)
                        e2 = wk.tile([128, CB * nh], F32, tag="e2")
                        nc.vector.tensor_tensor(out=e2[:], in0=e0[:], in1=e1[:],
                                                op=mybir.AluOpType.max)
                        exf = wk.tile([128, CB * nh], F32, tag="exf")
                        nc.scalar.activation(exf[:], e2[:],
                                             mybir.ActivationFunctionType.Exp)
                        ex = wk.tile([128, CB * nh], DT, tag="ex")
                        nc.vector.tensor_copy(ex[:], exf[:])
                        for j in range(CB):
                            b = c * CB + j
                            first, last = (b == 0), (b == NB - 1)
                            msg = wk.tile([128, hw], DT, tag="msg")
                            if l == 3:
                                nc.scalar.activation(
                                    msg[:, 0:ch], gt[:, j, 0:ch],
                                    mybir.ActivationFunctionType.Copy,
                                    scale=exf[:, j * nh:j * nh + 1])
                                if j % 2 == 0:
                                    nc.scalar.activation(
                                        msg[:, ch:2 * ch], gt[:, j, ch:2 * ch],
                                        mybir.ActivationFunctionType.Copy,
                                        scale=exf[:, j * nh + 1:j * nh + 2])
                                else:
                                    nc.vector.tensor_tensor(
                                        out=msg[:, ch:2 * ch],
                                        in0=gt[:, j, ch:2 * ch],
                                        in1=ex[:, j * nh + 1:j * nh + 2]
                                            .to_broadcast([128, ch]),
                                        op=mybir.AluOpType.mult)
                            else:
                                for h in range(2):
                                    nc.scalar.activation(
                                        msg[:, h * ch:(h + 1) * ch],
                                        gt[:, j, h * ch:(h + 1) * ch],
                                        mybir.ActivationFunctionType.Copy,
                                        scale=exf[:, j * nh + h:j * nh + h + 1])
                                nc.vector.tensor_tensor(
                                    out=msg[:, 2 * ch:hw].rearrange(
                                        "p (h c) -> p h c", h=nh - 2),
                                    in0=gt[:, j, 2 * ch:hw].rearrange(
                                        "p (h c) -> p h c", h=nh - 2),
                                    in1=ex[:, j * nh + 2:(j + 1) * nh]
                                        .to_broadcast([128, nh - 2, ch]),
                                    op=mybir.AluOpType.mult)
                            nc.tensor.matmul(p_den[:],
                                             lhsT=sel[:, j, :],
                                             rhs=ex[:, j * nh:(j + 1) * nh],
                                             start=first, stop=last)
                            if l == 3:
                                nc.tensor.matmul(p_num0[:], lhsT=sel[:, j, :],
                                                 rhs=msg[:, 0:512],
                                                 start=first, stop=last)
                                nc.tensor.matmul(p_num1[:], lhsT=sel[:, j, :],
                                                 rhs=msg[:, 512:1024],
                                                 start=first, stop=last)
                            else:
                                nc.tensor.matmul(p_num[:], lhsT=sel[:, j, :],
                                                 rhs=msg[:],
                                                 start=first, stop=last)
                    # tile epilogue
                    dn = wk.tile([128, nh], F32, tag="dn", bufs=1)
                    nc.vector.tensor_scalar_add(dn[:], p_den[:], EPS)
                    rc = wk.tile([128, nh], F32, tag="rc", bufs=1)
                    nc.vector.reciprocal(rc[:], dn[:])
                    if l == 3:
                        a0 = wk.tile([128, 512], F32, tag="a0", bufs=1)
                        nc.vector.tensor_tensor(
                            out=a0[:], in0=p_num0[:],
                            in1=rc[:, 0:1].to_broadcast([128, 512]),
                            op=mybir.AluOpType.mult)
                        a1 = wk.tile([128, 512], F32, tag="a1", bufs=1)
                        nc.vector.tensor_tensor(
                            out=a1[:], in0=p_num1[:],
                            in1=rc[:, 1:2].to_broadcast([128, 512]),
                            op=mybir.AluOpType.mult)
                        s0 = wk.tile([128, 512], F32, tag="s0", bufs=1)
                        nc.vector.tensor_tensor(out=s0[:], in0=a0[:], in1=a1[:],
                                                op=mybir.AluOpType.add)
                        s1 = wk.tile([128, 512], F32, tag="s1", bufs=1)
                        nc.vector.tensor_scalar(
                            out=s1[:], in0=s0[:], scalar1=0.5, scalar2=None,
                            op0=mybir.AluOpType.mult)
                        s2 = wk.tile([128, 512], F32, tag="s2", bufs=1)
                        nc.vector.tensor_tensor(out=s2[:], in0=s1[:],
                                                in1=x_prev(t),
                                                op=mybir.AluOpType.add)
                        s3 = wk.tile([128, 512], F32, tag="s3", bufs=1)
                        nc.vector.tensor_tensor(out=s3[:], in0=s2[:],
                                                in1=b_bc[3][:],
                                                op=mybir.AluOpType.add)
                        nc.sync.dma_start(t_out[t * 128:(t + 1) * 128, :], s3[:])
                    else:
                        agg = wk.tile([128, 512], F32, tag="agg", bufs=1)
                        nc.vector.tensor_tensor(
                            out=agg[:].rearrange("p (h c) -> p h c", h=nh),
                            in0=p_num[:].rearrange("p (h c) -> p h c", h=nh),
                            in1=rc[:].to_broadcast([128, nh, ch]),
                            op=mybir.AluOpType.mult)
                        s0 = wk.tile([128, 512], F32, tag="s0", bufs=1)
                        nc.vector.tensor_tensor(out=s0[:], in0=agg[:],
                                                in1=x_prev(t),
                                                op=mybir.AluOpType.add)
                        s1 = wk.tile([128, 512], F32, tag="s1", bufs=1)
                        nc.vector.tensor_tensor(out=s1[:], in0=s0[:],
                                                in1=b_bc[l][:],
                                                op=mybir.AluOpType.add)
                        # elu(x) = max(x,0) + exp(min(x,0)) - 1
                        mn = wk.tile([128, 512], F32, tag="mn", bufs=1)
                        nc.vector.tensor_scalar_min(mn[:], s1[:], 0.0)
                        ep = wk.tile([128, 512], F32, tag="ep", bufs=1)
                        nc.scalar.activation(ep[:], mn[:],
                                             mybir.ActivationFunctionType.Exp)
                        mx = wk.tile([128, 512], F32, tag="mx", bufs=1)
                        nc.vector.tensor_scalar(out=mx[:], in0=s1[:], scalar1=0.0,
                                                scalar2=-1.0,
                                                op0=mybir.AluOpType.max,
                                                op1=mybir.AluOpType.add)
                        nc.vector.tensor_tensor(out=x_next[:, t, :], in0=mx[:],
                                                in1=ep[:],
                                                op=mybir.AluOpType.add)

            # ---- layer 1 ---------------------------------------------------
            tbl1_in = dram.tile([NPC, WT12], DT, tag="t1in")
            tbl1 = dram.tile([NPAD, WT12], DT, tag="t1", addr_space="Shared")

            def lhsT_l1(nt):
                x_nt = wk.tile([128, 4, 128], DT, tag="xnt")
                nc.sync.dma_start(
                    x_nt[:],
                    t_xgT[:].rearrange("(kb p) n -> p kb n", p=128)
                        [:, :, nt * 128:(nt + 1) * 128])
                return x_nt

            build_table(1, tbl1_in, lhsT_l1)
            nc.gpsimd.collective_compute(
                "AllGather", mybir.AluOpType.bypass, replica_groups=rg,
                ins=[tbl1_in[:]], outs=[tbl1[:]])

            def xprev1(t):
                xp = wk.tile([128, 512], F32, tag="xp1", bufs=1)
                nc.sync.dma_start(xp[:], t_xl[t * 128:(t + 1) * 128, :])
                return xp[:]

            edge_phase(1, tbl1, tbl1_in, xprev1, xA)

            # ---- layers 2, 3 ----------------------------------------------
            def transpose_lhsT(x_buf):
                def get(nt):
                    xt = wk.tile([128, 4, 128], DT, tag="xnt")
                    for kb in range(4):
                        p_t = psml.tile([128, 128], F32, space="PSUM", tag="small")
                        nc.tensor.transpose(
                            out=p_t[:],
                            in_=x_buf[:, nt, kb * 128:(kb + 1) * 128],
                            identity=id_f32[:])
                        nc.scalar.activation(xt[:, kb, :], p_t[:],
                                             mybir.ActivationFunctionType.Copy)
                    return xt
                return get

            tbl2_in = dram.tile([NPC, WT12], DT, tag="t2in")
            tbl2 = dram.tile([NPAD, WT12], DT, tag="t2", addr_space="Shared")
            build_table(2, tbl2_in, transpose_lhsT(xA))
            nc.gpsimd.collective_compute(
                "AllGather", mybir.AluOpType.bypass, replica_groups=rg,
                ins=[tbl2_in[:]], outs=[tbl2[:]])
            edge_phase(2, tbl2, tbl2_in, lambda t: xA[:, t, :], xB)

            tbl3_in = dram.tile([NPC, WT3], DT, tag="t3in")
            tbl3 = dram.tile([NPAD, WT3], DT, tag="t3", addr_space="Shared")
            build_table(3, tbl3_in, transpose_lhsT(xB))
            nc.gpsimd.collective_compute(
                "AllGather", mybir.AluOpType.bypass, replica_groups=rg,
                ins=[tbl3_in[:]], outs=[tbl3[:]])
            edge_phase(3, tbl3, tbl3_in, lambda t: xB[:, t, :], None)

    nc.compile()
    return nc


def _run(inputs, trace=False):
    in_maps, NB = _prep_host(**inputs)
    key = NB
    if key not in _cache:
        _cache[key] = _build_nc(NB)
    nc = _cache[key]
    res = run_bass_kernel_spmd(nc, in_maps, core_ids=list(range(NCORES)),
                               trace=trace)
    out = np.concatenate([r["out"] for r in res.results], axis=0)[:N]
    return out, res


def kernel(**inputs):
    out, _ = _run(inputs, trace=False)
    return out


# revision 13
# speedup vs baseline: 1.2075x; 1.0224x over previous
"""3-layer GAT (graph attention network) on 8 Trainium2 NeuronCores.

Strategy: node-sharded graph parallelism.
- Nodes padded 10000 -> 10240, 1280 per core; edges partitioned by dst range.
- Per layer: each core computes table rows [h | es | ed] for its nodes with a
  PE matmul against W_ext = [W | W@Bsrc | W@Bdst] (bf16), AllGathers the full
  table, then processes its edges: dma_gather of h[src] rows, attention
  softmax without segment-max (exponents are bounded, softmax is shift
  invariant), and scatter-by-matmul: sel[e,dst] one-hot matrices contract
  128-edge blocks into per-dst-tile PSUM accumulators for both the
  numerator (sum alpha*h) and denominator (sum exp e).
"""

import numpy as np
import ml_dtypes

import concourse.bass as bass
import concourse.bacc as bacc
import concourse.mybir as mybir
import concourse.tile as tile
from concourse.library_config import mlp
from concourse.masks import make_identity
from concourse.bass_utils import run_bass_kernel_spmd
from concourse._compat import cdiv

F32 = mybir.dt.float32
DT = mybir.dt.bfloat16
NPDT = ml_dtypes.bfloat16

N, E, D = 10000, 160000, 512
H, C = 4, 128
HF, CF = 2, 512
NEG = 0.2
EPS = 1e-16

NCORES = 8
NPAD = 10240
NPC = NPAD // NCORES       # 1280 nodes per core
NTL = NPC // 128           # 10 local dst tiles per core
NTG = NPAD // 128          # 80 global node tiles
CB = 6                     # gather-chunk size in 128-edge blocks (768 idxs)
WT12 = 640                 # bf16 table row width, layers 1-2 (512+4+4 -> pad)
WT3 = 1152                 # layer 3 (1024+2+2 -> pad)

_cache = {}


def _block_diag(a):
    """[Hh, Cc] attention vector -> [Hh*Cc, Hh] block-diagonal embed."""
    Hh, Cc = a.shape
    B = np.zeros((Hh * Cc, Hh), np.float32)
    for h in range(Hh):
        B[h * Cc:(h + 1) * Cc, h] = a[h]
    return B


def _prep_host(graph, edge_index, W1, as1, ad1, b1, W2, as2, ad2, b2,
               W3, as3, ad3, b3):
    src = np.asarray(edge_index[0], np.int64)
    dst = np.asarray(edge_index[1], np.int64)

    dstt = dst // 128
    cnt = np.bincount(dstt, minlength=NTG)
    nb = int(np.ceil(cnt.max() / 128))
    NB = max(18, int(cdiv(nb, CB)) * CB)          # blocks per dst tile
    NCHUNK = NB // CB
    order = np.argsort(dstt, kind="stable")
    off = np.concatenate([[0], np.cumsum(cnt)])

    idx_slots = np.zeros((NTG, NB * 128), np.int16)
    dstl_slots = np.full((NTG, NB * 128), 255.0, np.float32)
    for gt in range(NTG):
        e = order[off[gt]:off[gt + 1]]
        k = len(e)
        idx_slots[gt, :k] = src[e].astype(np.int16)
        dstl_slots[gt, :k] = (dst[e] - gt * 128).astype(np.float32)

    # dma_gather wrapped index layout: within each 768-idx chunk,
    # unwrapped[j] = wrapped[j % 16, j // 16]; replicated to 128 partitions.
    w = idx_slots.reshape(NTG, NCHUNK, 48, 16).transpose(0, 1, 3, 2)  # [.,.,16,48]
    w = np.tile(w, (1, 1, 8, 1))                                      # [.,.,128,48]
    # selection matrices, shared across layers (depend only on edge layout):
    # sel[p, (t,c,j,d)]: slot (c*6+j)*128+p of tile t targets local dst d
    # selT[d, (t,c,j,e)]: transpose
    oh = (dstl_slots[:, :, None] ==
          np.arange(128, dtype=np.float32)[None, None, :])            # [NTG,NB*128,128]
    oh = oh.reshape(NTG, NCHUNK, CB, 128, 128)                        # [t,c,j,p,d]
    sel_h = oh.transpose(0, 1, 3, 2, 4).astype(NPDT)                  # [t,c,p,j,d]
    selT_h = oh.transpose(0, 1, 4, 2, 3).astype(NPDT)                 # [t,c,d,j,e]
    dcol = dstl_slots.reshape(NTG, NB, 128).transpose(0, 2, 1)        # [NTG,128,NB]

    xpad = np.zeros((NPAD, D), np.float32)
    xpad[:N] = np.asarray(graph, np.float32)

    def wext(W, a_s, a_d, wt):
        cols = np.concatenate(
            [W, W @ _block_diag(a_s), W @ _block_diag(a_d)], axis=1)
        out = np.zeros((D, wt), np.float32)
        out[:, :cols.shape[1]] = cols
        return out.astype(NPDT)

    we1 = wext(np.asarray(W1, np.float32), np.asarray(as1), np.asarray(ad1), WT12)
    we2 = wext(np.asarray(W2, np.float32), np.asarray(as2), np.asarray(ad2), WT12)
    we3 = wext(np.asarray(W3, np.float32), np.asarray(as3), np.asarray(ad3), WT3)

    ones_row = np.ones((1, 128), np.float32)
    iotaF6 = np.tile(np.arange(128, dtype=np.float32)[None, :], (128, 6)).astype(NPDT)
    iotaP = np.arange(128, dtype=np.float32)[:, None].astype(NPDT)

    in_maps = []
    for c in range(NCORES):
        tl = slice(c * NTL, (c + 1) * NTL)
        idx_c = w[tl].transpose(2, 0, 1, 3).reshape(128, NTL * NCHUNK * 48)
        sel_c = sel_h[tl].transpose(2, 0, 1, 3, 4).reshape(128, NTL * NCHUNK * 768)
        selT_c = selT_h[tl].transpose(2, 0, 1, 3, 4).reshape(128, NTL * NCHUNK * 768)
        dstl_c = dcol[tl].transpose(1, 0, 2).reshape(128, NTL * NB).astype(NPDT)
        dstlb_c = np.tile(dstl_slots[tl].reshape(1, NTL * NB * 128),
                          (128, 1)).astype(NPDT)
        xgT_c = np.ascontiguousarray(
            xpad[c * NPC:(c + 1) * NPC].T).astype(NPDT)   # [512, 1280]
        xl_c = xpad[c * NPC:(c + 1) * NPC]                 # [1280, 512] f32
        in_maps.append({
            "idx": np.ascontiguousarray(idx_c),
            "selh": np.ascontiguousarray(sel_c),
            "selTh": np.ascontiguousarray(selT_c),
            "dstl": np.ascontiguousarray(dstl_c),
            "dstlb": np.ascontiguousarray(dstlb_c),
            "xgT": xgT_c,
            "xl": np.ascontiguousarray(xl_c),
            "we1": we1, "we2": we2, "we3": we3,
            "b1": np.asarray(b1, np.float32)[None, :],
            "b2": np.asarray(b2, np.float32)[None, :],
            "b3": np.asarray(b3, np.float32)[None, :],
            "ones": ones_row,
            "iotaF6": iotaF6,
            "iotaP": iotaP,
        })
    return in_maps, NB


def _build_nc(NB):
    NCHUNK = NB // CB
    nc = bacc.Bacc("TRN2", target_bir_lowering=False, debug=False,
                   num_devices=NCORES, num_swdge_queues=4)

    t_idx = nc.dram_tensor("idx", [128, NTL * NCHUNK * 48], mybir.dt.int16,
                           kind="ExternalInput")
    t_selh = nc.dram_tensor("selh", [128, NTL * NCHUNK * 768], DT,
                            kind="ExternalInput")
    t_selTh = nc.dram_tensor("selTh", [128, NTL * NCHUNK * 768], DT,
                             kind="ExternalInput")
    t_dstl = nc.dram_tensor("dstl", [128, NTL * NB], DT, kind="ExternalInput")
    t_dstlb = nc.dram_tensor("dstlb", [128, NTL * NB * 128], DT,
                             kind="ExternalInput")
    t_iotaF6 = nc.dram_tensor("iotaF6", [128, 768], DT, kind="ExternalInput")
    t_iotaP = nc.dram_tensor("iotaP", [128, 1], DT, kind="ExternalInput")
    t_xgT = nc.dram_tensor("xgT", [D, NPC], DT, kind="ExternalInput")
    t_xl = nc.dram_tensor("xl", [NPC, D], F32, kind="ExternalInput")
    t_we = {1: nc.dram_tensor("we1", [D, WT12], DT, kind="ExternalInput"),
            2: nc.dram_tensor("we2", [D, WT12], DT, kind="ExternalInput"),
            3: nc.dram_tensor("we3", [D, WT3], DT, kind="ExternalInput")}
    t_b = {1: nc.dram_tensor("b1", [1, D], F32, kind="ExternalInput"),
           2: nc.dram_tensor("b2", [1, D], F32, kind="ExternalInput"),
           3: nc.dram_tensor("b3", [1, D], F32, kind="ExternalInput")}

    t_ones = nc.dram_tensor("ones", [1, 128], F32, kind="ExternalInput")
    t_out = nc.dram_tensor("out", [NPC, D], F32, kind="ExternalOutput")

    rg = [list(range(NCORES))]
    qn = [0]

    with tile.TileContext(nc) as tc:
        with tc.tile_pool(name="cst", bufs=1) as cst, \
             tc.tile_pool(name="per", bufs=1) as per, \
             tc.tile_pool(name="wk", bufs=2) as wk, \
             tc.tile_pool(name="ed", bufs=2) as edp, \
             tc.tile_pool(name="gath", bufs=4) as gp, \
             tc.tile_pool(name="pnum", bufs=2, space="PSUM") as pnum, \
             tc.tile_pool(name="pden", bufs=2, space="PSUM") as pden, \
             tc.tile_pool(name="psmall", bufs=2, space="PSUM") as psml, \
             tc.tile_pool(name="dram", bufs=1, space="DRAM") as dram:

            nc.gpsimd.load_library(mlp)

            # ---- constants -------------------------------------------------
            id_f32 = cst.tile([128, 128], F32)
            make_identity(nc, id_f32[:])
            iotaF6 = cst.tile([128, 768], DT)
            nc.sync.dma_start(iotaF6[:], t_iotaF6[:])
            iotaP = cst.tile([128, 1], DT)
            nc.sync.dma_start(iotaP[:], t_iotaP[:])
            dstl = per.tile([128, NTL * NB], DT)
            nc.sync.dma_start(dstl[:], t_dstl[:])
            ones_t = cst.tile([1, 128], F32)
            nc.sync.dma_start(ones_t[:], t_ones[:])

            idx_t = per.tile([128, NTL * NCHUNK * 48], mybir.dt.int16)
            nc.sync.dma_start(idx_t[:], t_idx[:])

            b_bc = {}
            for l in (1, 2, 3):
                br = wk.tile([1, D], F32, tag="brow", bufs=1)
                nc.sync.dma_start(br[:], t_b[l][:])
                pb = pnum.tile([128, D], F32, space="PSUM", tag="numA")
                nc.tensor.matmul(pb[:], lhsT=ones_t[:], rhs=br[:],
                                 start=True, stop=True)
                b_bc[l] = cst.tile([128, D], F32, tag=f"bbc{l}", name=f"bbc{l}")
                nc.vector.tensor_copy(b_bc[l][:], pb[:])

            we = {}
            for l in (1, 2, 3):
                wt = WT3 if l == 3 else WT12
                we[l] = per.tile([128, 4, wt], DT, tag=f"we{l}", name=f"we{l}")
                nc.sync.dma_start(
                    we[l][:],
                    t_we[l][:].rearrange("(kb p) w -> p kb w", p=128))

            # persistent x buffers (f32) for layer 2/3 inputs
            xA = per.tile([128, NTL, D], F32, tag="xA")
            xB = per.tile([128, NTL, D], F32, tag="xB")

            # ---- per-layer helpers ----------------------------------------
            def build_table(l, tbl_in, get_lhsT):
                """Local table rows: [h | es | ed] for this core's nodes."""
                wt = WT3 if l == 3 else WT12
                hw = HF * CF if l == 3 else H * C
                nh = HF if l == 3 else H
                segs = [(0, 512), (512, 1024), (1024, 1024 + 2 * nh)] if l == 3 \
                    else [(0, 512), (512, 512 + 2 * nh)]
                for nt in range(NTL):
                    lhsT = get_lhsT(nt)  # [128, 4, 128] DT tile
                    psums = []
                    for si, (c0, c1) in enumerate(segs):
                        if c1 - c0 > 64:
                            pool_, tag = pnum, ("numA" if si == 0 else "numB")
                        else:
                            pool_, tag = psml, "small"
                        p = pool_.tile([128, c1 - c0], F32, space="PSUM",
                                       tag=tag, name=f"p_tbl{si}")
                        for kb in range(4):
                            nc.tensor.matmul(p[:], lhsT=lhsT[:, kb, :],
                                             rhs=we[l][:, kb, c0:c1],
                                             start=(kb == 0), stop=(kb == 3))
                        psums.append((c0, c1, p))
                    row = wk.tile([128, wt], DT, tag="tblrow")
                    for c0, c1, p in psums:
                        nc.vector.tensor_copy(row[:, c0:c1], p[:])
                    nc.sync.dma_start(tbl_in[nt * 128:(nt + 1) * 128, :], row[:])

            def edge_phase(l, tbl, tbl_in, x_prev, x_next):
                wt = WT3 if l == 3 else WT12
                nh = HF if l == 3 else H
                ch = CF if l == 3 else C
                hw = nh * ch
                es_off, ed_off = hw, hw + nh
                for t in range(NTL):
                    if l == 3:
                        dstlb_t = wk.tile([128, NB * 128], DT, tag="dstlb")
                        nc.sync.dma_start(
                            dstlb_t[:],
                            t_dstlb[:, t * NB * 128:(t + 1) * NB * 128])
                    ed_t = edp.tile([128, nh], DT, tag="edt")
                    nc.sync.dma_start(
                        ed_t[:],
                        tbl_in[t * 128:(t + 1) * 128, ed_off:ed_off + nh])
                    if l == 3:
                        p_num0 = pnum.tile([128, 512], F32, space="PSUM", tag="numA")
                        p_num1 = pnum.tile([128, 512], F32, space="PSUM", tag="numB")
                    else:
                        p_num = pnum.tile([128, 512], F32, space="PSUM", tag="numA")
                    p_den = pden.tile([128, nh], F32, space="PSUM", tag="den")
                    for c in range(NB // CB):
                        gt = gp.tile([128, CB, wt], DT, tag="gt")
                        icol = (t * (NB // CB) + c) * 48
                        nc.gpsimd.dma_gather(
                            gt[:], tbl[:], idx_t[:, icol:icol + 48],
                            CB * 128, CB * 128, wt, queue_num=qn[0] % 4)
                        qn[0] += 1
                        scol = (t * NCHUNK + c) * 768
                        selT = wk.tile([128, CB * 128], DT, tag="selT", bufs=3)
                        sel = wk.tile([128, CB, 128], DT, tag="sel", bufs=3)
                        if l == 3:
                            nc.vector.tensor_tensor(
                                out=selT[:],
                                in0=iotaP[:].to_broadcast([128, CB * 128]),
                                in1=dstlb_t[:, c * CB * 128:(c + 1) * CB * 128],
                                op=mybir.AluOpType.is_equal)
                            dc0 = t * NB + c * CB
                            nc.vector.tensor_tensor(
                                out=sel[:],
                                in0=dstl[:, dc0:dc0 + CB, None].to_broadcast(
                                    [128, CB, 128]),
                                in1=iotaF6[:].rearrange("p (b f) -> p b f", b=CB),
                                op=mybir.AluOpType.is_equal)
                        else:
                            nc.sync.dma_start(selT[:],
                                              t_selTh[:, scol:scol + 768])
                            nc.sync.dma_start(
                                sel[:], t_selh[:, scol:scol + 768]
                                .rearrange("p (b f) -> p b f", b=CB))
                        p_ede = psml.tile([128, CB * nh], F32, space="PSUM",
                                          tag="small")
                        for j in range(CB):
                            nc.tensor.matmul(
                                p_ede[:, j * nh:(j + 1) * nh],
                                lhsT=selT[:, j * 128:(j + 1) * 128],
                                rhs=ed_t[:], start=True, stop=True)
                        e0 = wk.tile([128, CB * nh], F32, tag="e0")
                        nc.vector.tensor_tensor(
                            out=e0[:].rearrange("p (b h) -> p b h", b=CB),
                            in0=gt[:, :, es_off:es_off + nh],
                            in1=p_ede[:].rearrange("p (b h) -> p b h", b=CB),
                            op=mybir.AluOpType.add)
                        e1 = wk.tile([128, CB * nh], F32, tag="e1")
                        nc.vector.tensor_scalar_mul(e1[:], e0[:], NEG)
                        e2 = wk.tile([128, CB * nh], F32, tag="e2")
                        nc.vector.tensor_tensor(out=e2[:], in0=e0[:], in1=e1[:],
                                                op=mybir.AluOpType.max)
                        exf = wk.tile([128, CB * nh], F32, tag="exf")
                        nc.scalar.activation(exf[:], e2[:],
                                             mybir.ActivationFunctionType.Exp)
                        ex = wk.tile([128, CB * nh], DT, tag="ex")
                        nc.vector.tensor_copy(ex[:], exf[:])
                        for j in range(CB):
                            b = c * CB + j
                            first, last = (b == 0), (b == NB - 1)
                            msg = wk.tile([128, hw], DT, tag="msg")
                            if l == 3:
                                nc.scalar.activation(
                                    msg[:, 0:ch], gt[:, j, 0:ch],
                                    mybir.ActivationFunctionType.Copy,
                                    scale=exf[:, j * nh:j * nh + 1])
                                if j % 2 == 0:
                                    nc.scalar.activation(
                                        msg[:, ch:2 * ch], gt[:, j, ch:2 * ch],
                                        mybir.ActivationFunctionType.Copy,
                                        scale=exf[:, j * nh + 1:j * nh + 2])
                                else:
                                    nc.vector.tensor_tensor(
                                        out=msg[:, ch:2 * ch],
                                        in0=gt[:, j, ch:2 * ch],
                                        in1=ex[:, j * nh + 1:j * nh + 2]
                                            .to_broadcast([128, ch]),
                                        op=mybir.AluOpType.mult)
                            else:
                                for h in range(2):
                                    nc.scalar.activation(
                                        msg[:, h * ch:(h + 1) * ch],
                                        gt[:, j, h * ch:(h + 1) * ch],
                                        mybir.ActivationFunctionType.Copy,
                                        scale=exf[:, j * nh + h:j * nh + h + 1])
                                nc.vector.tensor_tensor(
                                    out=msg[:, 2 * ch:hw].rearrange(
                                        "p (h c) -> p h c", h=nh - 2),
                                    in0=gt[:, j, 2 * ch:hw].rearrange(
                                        "p (h c) -> p h c", h=nh - 2),
                                    in1=ex[:, j * nh + 2:(j + 1) * nh]
                                        .to_broadcast([128, nh - 2, ch]),
                                    op=mybir.AluOpType.mult)
                            nc.tensor.matmul(p_den[:],
                                             lhsT=sel[:, j, :],
                                             rhs=ex[:, j * nh:(j + 1) * nh],
                                             start=first, stop=last)
                            if l == 3:
                                nc.tensor.matmul(p_num0[:], lhsT=sel[:, j, :],
                                                 rhs=msg[:, 0:512],
                                                 start=first, stop=last)
                                nc.tensor.matmul(p_num1[:], lhsT=sel[:, j, :],
                                                 rhs=msg[:, 512:1024],
                                                 start=first, stop=last)
                            else:
                                nc.tensor.matmul(p_num[:], lhsT=sel[:, j, :],
                                                 rhs=msg[:],
                                                 start=first, stop=last)
                    # tile epilogue
                    dn = wk.tile([128, nh], F32, tag="dn", bufs=1)
                    nc.vector.tensor_scalar_add(dn[:], p_den[:], EPS)
                    rc = wk.tile([128, nh], F32, tag="rc", bufs=1)
                    nc.vector.reciprocal(rc[:], dn[:])
                    if l == 3:
                        a0 = wk.tile([128, 512], F32, tag="a0", bufs=1)
                        nc.vector.tensor_tensor(
                            out=a0[:], in0=p_num0[:],
                            in1=rc[:, 0:1].to_broadcast([128, 512]),
                            op=mybir.AluOpType.mult)
                        a1 = wk.tile([128, 512], F32, tag="a1", bufs=1)
                        nc.vector.tensor_tensor(
                            out=a1[:], in0=p_num1[:],
                            in1=rc[:, 1:2].to_broadcast([128, 512]),
                            op=mybir.AluOpType.mult)
                        s0 = wk.tile([128, 512], F32, tag="s0", bufs=1)
                        nc.vector.tensor_tensor(out=s0[:], in0=a0[:], in1=a1[:],
                                                op=mybir.AluOpType.add)
                        s1 = wk.tile([128, 512], F32, tag="s1", bufs=1)
                        nc.vector.tensor_scalar(
                            out=s1[:], in0=s0[:], scalar1=0.5, scalar2=None,
                            op0=mybir.AluOpType.mult)
                        s2 = wk.tile([128, 512], F32, tag="s2", bufs=1)
                        nc.vector.tensor_tensor(out=s2[:], in0=s1[:],
                                                in1=x_prev(t),
                                                op=mybir.AluOpType.add)
                        s3 = wk.tile([128, 512], F32, tag="s3", bufs=1)
                        nc.vector.tensor_tensor(out=s3[:], in0=s2[:],
                                                in1=b_bc[3][:],
                                                op=mybir.AluOpType.add)
                        nc.sync.dma_start(t_out[t * 128:(t + 1) * 128, :], s3[:])
                    else:
                        agg = wk.tile([128, 512], F32, tag="agg", bufs=1)
                        nc.vector.tensor_tensor(
                            out=agg[:].rearrange("p (h c) -> p h c", h=nh),
                            in0=p_num[:].rearrange("p (h c) -> p h c", h=nh),
                            in1=rc[:].to_broadcast([128, nh, ch]),
                            op=mybir.AluOpType.mult)
                        s0 = wk.tile([128, 512], F32, tag="s0", bufs=1)
                        nc.vector.tensor_tensor(out=s0[:], in0=agg[:],
                                                in1=x_prev(t),
                                                op=mybir.AluOpType.add)
                        s1 = wk.tile([128, 512], F32, tag="s1", bufs=1)
                        nc.vector.tensor_tensor(out=s1[:], in0=s0[:],
                                                in1=b_bc[l][:],
                                                op=mybir.AluOpType.add)
                        # elu(x) = max(x,0) + exp(min(x,0)) - 1
                        mn = wk.tile([128, 512], F32, tag="mn", bufs=1)
                        nc.vector.tensor_scalar_min(mn[:], s1[:], 0.0)
                        ep = wk.tile([128, 512], F32, tag="ep", bufs=1)
                        nc.scalar.activation(ep[:], mn[:],
                                             mybir.ActivationFunctionType.Exp)
                        mx = wk.tile([128, 512], F32, tag="mx", bufs=1)
                        nc.vector.tensor_scalar(out=mx[:], in0=s1[:], scalar1=0.0,
                                                scalar2=-1.0,
                                                op0=mybir.AluOpType.max,
                                                op1=mybir.AluOpType.add)
                        nc.vector.tensor_tensor(out=x_next[:, t, :], in0=mx[:],
                                                in1=ep[:],
                                                op=mybir.AluOpType.add)

            # ---- layer 1 ---------------------------------------------------
            tbl1_in = dram.tile([NPC, WT12], DT, tag="t1in")
            tbl1 = dram.tile([NPAD, WT12], DT, tag="t1", addr_space="Shared")

            def lhsT_l1(nt):
                x_nt = wk.tile([128, 4, 128], DT, tag="xnt")
                nc.sync.dma_start(
                    x_nt[:],
                    t_xgT[:].rearrange("(kb p) n -> p kb n", p=128)
                        [:, :, nt * 128:(nt + 1) * 128])
                return x_nt

            build_table(1, tbl1_in, lhsT_l1)
            nc.gpsimd.collective_compute(
                "AllGather", mybir.AluOpType.bypass, replica_groups=rg,
                ins=[tbl1_in[:]], outs=[tbl1[:]])

            def xprev1(t):
                xp = wk.tile([128, 512], F32, tag="xp1", bufs=1)
                nc.sync.dma_start(xp[:], t_xl[t * 128:(t + 1) * 128, :])
                return xp[:]

            edge_phase(1, tbl1, tbl1_in, xprev1, xA)

            # ---- layers 2, 3 ----------------------------------------------
            def transpose_lhsT(x_buf):
                def get(nt):
                    xt = wk.tile([128, 4, 128], DT, tag="xnt")
                    for kb in range(4):
                        p_t = psml.tile([128, 128], F32, space="PSUM", tag="small")
                        nc.tensor.transpose(
                            out=p_t[:],
                            in_=x_buf[:, nt, kb * 128:(kb + 1) * 128],
                            identity=id_f32[:])
                        nc.scalar.activation(xt[:, kb, :], p_t[:],
                                             mybir.ActivationFunctionType.Copy)
                    return xt
                return get

            tbl2_in = dram.tile([NPC, WT12], DT, tag="t2in")
            tbl2 = dram.tile([NPAD, WT12], DT, tag="t2", addr_space="Shared")
            build_table(2, tbl2_in, transpose_lhsT(xA))
            nc.gpsimd.collective_compute(
                "AllGather", mybir.AluOpType.bypass, replica_groups=rg,
                ins=[tbl2_in[:]], outs=[tbl2[:]])
            edge_phase(2, tbl2, tbl2_in, lambda t: xA[:, t, :], xB)

            tbl3_in = dram.tile([NPC, WT3], DT, tag="t3in")
            tbl3 = dram.tile([NPAD, WT3], DT, tag="t3", addr_space="Shared")
            build_table(3, tbl3_in, transpose_lhsT(xB))
            nc.gpsimd.collective_compute(
                "AllGather", mybir.AluOpType.bypass, replica_groups=rg,
                ins=[tbl3_in[:]], outs=[tbl3[:]])
            edge_phase(3, tbl3, tbl3_in, lambda t: xB[:, t, :], None)

    nc.compile()
    return nc


def _run(inputs, trace=False):
    in_maps, NB = _prep_host(**inputs)
    key = NB
    if key not in _cache:
        _cache[key] = _build_nc(NB)
    nc = _cache[key]
    res = run_bass_kernel_spmd(nc, in_maps, core_ids=list(range(NCORES)),
                               trace=trace)
    out = np.concatenate([r["out"] for r in res.results], axis=0)[:N]
    return out, res


def kernel(**inputs):
    out, _ = _run(inputs, trace=False)
    return out
